# revision 1
# baseline (speedup 1.0000x reference)
"""Deformable transformer encoder layer on 8 TRN2 NeuronCores.

Sharding: core c -> (batch b=c//4, quarter s=c%4) of the 19560 query tokens
(padded to 4992 = 39*128). Each core computes the full-batch value table
(redundantly, avoids collectives), writes it to DRAM as [head, pixel, 32]
fp16, and bilinearly gathers x-pixel-pairs (64 contiguous fp16 = 128B) with
indirect DMA at 32-element (pixel) granularity. Bilinear corner weights are
computed feature-major (128 partitions = (head, level, point)) and include
border-validity zeroing, so no table padding is needed. Matmuls run fp16 x
fp16 -> fp32 PSUM; coordinates/indices and LN run fp32.
"""
import sys, os
sys.path.insert(0, "/opt/trn_rl_repo")

import numpy as np
from contextlib import ExitStack

import concourse.bass as bass
import concourse.tile as tile
from concourse import bacc, mybir
from concourse.masks import make_identity

FP32 = mybir.dt.float32
FP16 = mybir.dt.float16
I32 = mybir.dt.int32
AX = mybir.AxisListType
OP = mybir.AluOpType
AF = mybir.ActivationFunctionType

SHAPES = [(92, 160), (46, 80), (23, 40), (12, 20)]
NH, HD, NL, NP = 8, 32, 4, 4
D = NH * HD
DF = 4 * D
EPS = 1e-5
B = 2
NCORES = 8
NSHARD = 4
FMT = 1024  # FM-block token width


def _plan(shapes):
    L = sum(h * w for h, w in shapes)
    lvl_start = np.cumsum([0] + [h * w for h, w in shapes])[:-1].tolist()
    Q = (L + NSHARD - 1) // NSHARD
    QP = ((Q + 127) // 128) * 128
    LP = ((L + 127) // 128) * 128
    return L, lvl_start, Q, QP, LP


L, LVL_START, Q, QP, LP = _plan(SHAPES)    # 19560, ..., 4890, 4992, 19584


def _fm_blocks(T, w=FMT):
    out, t0 = [], 0
    while t0 < T:
        out.append((t0, min(w, T - t0)))
        t0 += w
    return out


def build_program(shapes=SHAPES):
    Lx, lvl_start, Qx, QPx, LPx = _plan(shapes)
    NPIX = NH * LPx

    nc = bacc.Bacc("TRN2", target_bir_lowering=False, debug=False,
                   enable_asserts=False, num_devices=1)

    def din(name, shape, dt=FP32):
        return nc.dram_tensor(name, list(shape), dt, kind="ExternalInput").ap()

    io = {
        "src_full": din("src_full", [LPx, D]),
        "srcq": din("srcq", [QPx, D]),
        "posq": din("posq", [QPx, D]),
        "refx_fm": din("refx_fm", [128, QPx]),
        "refy_fm": din("refy_fm", [128, QPx]),
        "Wvh": din("Wvh", [D, D], FP16),
        "WoffPh": din("WoffPh", [D, D], FP16),
        "Wah": din("Wah", [D, 128], FP16),
        "Woh": din("Woh", [D, D], FP16),
        "W1h": din("W1h", [D, DF], FP16),
        "W2h": din("W2h", [DF, D], FP16),
        "b1c": din("b1c", [128, DF // 128]),
        "ba_f": din("ba_f", [128, 1]),
        "boffP": din("boffP", [128, 2]),
        "fconst": din("fconst", [128, 7]),
        "hsum": din("hsum", [128, NH]),
        "hsumT": din("hsumT", [NH, 128]),
        "g1": din("g1", [D]), "be1": din("be1", [D]),
        "g2": din("g2", [D]), "be2": din("be2", [D]),
    }
    out_ap = nc.dram_tensor("out", [QPx, D], FP32, kind="ExternalOutput").ap()
    dbg = {}
    if os.environ.get("KDBG"):
        dbg = {
            "d_idx": nc.dram_tensor("d_idx", [8, 128, 256], I32, kind="ExternalOutput").ap(),
            "d_w4": nc.dram_tensor("d_w4", [8, 128, 512], FP16, kind="ExternalOutput").ap(),
            "d_g2": nc.dram_tensor("d_g2", [8, 128, 4096], FP16, kind="ExternalOutput").ap(),
            "d_ao": nc.dram_tensor("d_ao", [8, 128, 256], FP32, kind="ExternalOutput").ap(),
            "d_x1": nc.dram_tensor("d_x1", [8, 128, 256], FP32, kind="ExternalOutput").ap(),
            "d_off": nc.dram_tensor("d_off", [2, 128, 1024], FP32, kind="ExternalOutput").ap(),
            "d_afm": nc.dram_tensor("d_afm", [128, 1024], FP16, kind="ExternalOutput").ap(),
            "d_it": nc.dram_tensor("d_it", [2, 128, 1024], FP32, kind="ExternalOutput").ap(),
            "d_vtab": nc.dram_tensor("d_vtab", [2048, 32], FP16, kind="ExternalOutput").ap(),
        }
    vtab = nc.dram_tensor("vtab", [NPIX + 256, HD], FP16).ap()
    vtabS = nc.dram_tensor("vtabS", [NH * Lx, 4 * HD], FP16).ap()
    x1n_dram = nc.dram_tensor("x1n_scratch", [QPx, D], FP32).ap()

    with tile.TileContext(nc) as tc:
        with ExitStack() as ctx:
            _body(ctx, tc, io, out_ap, vtab, vtabS, x1n_dram,
                  shapes, lvl_start, Lx, QPx, LPx, dbg)
    nc.compile()
    return nc


def _body(ctx, tc, io, out_ap, vtab, vtabS, x1n_dram, shapes, lvl_start, Lx, QPx, LPx, dbg={}):
    nc = tc.nc
    KC = D // 128
    MF = DF // 128

    consts = ctx.enter_context(tc.tile_pool(name="consts", bufs=1))
    vpool = ctx.enter_context(tc.tile_pool(name="vpool", bufs=2))
    vtmp = ctx.enter_context(tc.tile_pool(name="vtmp", bufs=2))
    fmp = ctx.enter_context(tc.tile_pool(name="fmp", bufs=1))
    chain = ctx.enter_context(tc.tile_pool(name="chain", bufs=1))
    blk = ctx.enter_context(tc.tile_pool(name="blk", bufs=2))
    gpool = ctx.enter_context(tc.tile_pool(name="gpool", bufs=2))
    psTP = ctx.enter_context(tc.tile_pool(name="psTP", bufs=4, space="PSUM"))
    psMM = ctx.enter_context(tc.tile_pool(name="psMM", bufs=3, space="PSUM"))
    psS = ctx.enter_context(tc.tile_pool(name="psS", bufs=1, space="PSUM"))

    # ---- constants ----
    ident = consts.tile([128, 128], FP32)
    make_identity(nc, ident[:])

    def wtiles(name, ap, ncol):
        ts = []
        for k in range(ap.shape[0] // 128):
            t = consts.tile([128, ncol], FP16, tag=f"{name}{k}")
            nc.sync.dma_start(out=t[:], in_=ap[k * 128:(k + 1) * 128, :])
            ts.append(t)
        return ts

    Wv_s = wtiles("Wv", io["Wvh"], D)
    Woff_s = wtiles("Woff", io["WoffPh"], D)
    Wa_s = wtiles("Wa", io["Wah"], 128)
    Wo_s = wtiles("Wo", io["Woh"], D)
    W1_s = wtiles("W1", io["W1h"], DF)
    W2_s = wtiles("W2", io["W2h"], D)

    def rep_tile(name, ap):
        t = consts.tile([128, D], FP32, tag=name)
        bap = bass.AP(tensor=ap.tensor, offset=ap.offset, ap=[[0, 128], [1, D]])
        nc.gpsimd.dma_start(out=t[:], in_=bap)
        return t

    g1_rep = rep_tile("g1_rep", io["g1"])
    be1_rep = rep_tile("be1_rep", io["be1"])
    g2_rep = rep_tile("g2_rep", io["g2"])
    be2_rep = rep_tile("be2_rep", io["be2"])

    def load(name, ap, dt=FP32):
        t = consts.tile(list(ap.shape), dt, tag=name)
        nc.sync.dma_start(out=t[tuple(slice(0, s) for s in ap.shape)],
                          in_=ap[tuple(slice(None) for _ in ap.shape)])
        return t

    eps_t = consts.tile([128, 1], FP32, tag="eps_t")
    nc.vector.memset(eps_t[:], EPS)
    b1c = load("b1c", io["b1c"])
    ba_f = load("ba_f", io["ba_f"])
    boffP = load("boffP", io["boffP"])
    fconst = load("fconst", io["fconst"])
    hsum = load("hsum", io["hsum"])
    hsumT = load("hsumT", io["hsumT"])
    Wf, Hf = fconst[:, 0:1], fconst[:, 1:2]
    Wm1, Hm1 = fconst[:, 2:3], fconst[:, 3:4]
    basef = fconst[:, 4:5]
    Wm2, Hm2 = fconst[:, 5:6], fconst[:, 6:7]

    def tp128(src_ap, dst_ap, engine=None):
        assert src_ap.dtype == FP32
        ps = psTP.tile([128, 128], FP32, tag="tp", name="tp")
        nc.tensor.transpose(out=ps[:], in_=src_ap, identity=ident[:])
        if engine is nc.vector:
            nc.vector.tensor_copy(out=dst_ap, in_=ps[:])
        else:
            nc.scalar.copy(out=dst_ap, in_=ps[:])

    # ================= Phase V: value table =================
    for sup in range(0, LPx, 1024):
        ntile = min(8, (LPx - sup) // 128)
        v_s = vpool.tile([128, 8, D], FP16, tag="v_s")
        for j in range(ntile):
            t0 = sup + j * 128
            s_t = vtmp.tile([128, D], FP32, tag="s_t")
            nc.sync.dma_start(out=s_t[:], in_=io["src_full"][t0:t0 + 128, :])
            sFM = vtmp.tile([128, KC, 128], FP16, tag="sFM")
            for k in range(KC):
                tp128(s_t[:, k * 128:(k + 1) * 128], sFM[:, k, :])
            psV = psMM.tile([128, 512], FP32, tag="mm")
            for k in range(KC):
                nc.tensor.matmul(psV[:, :D], lhsT=sFM[:, k, :], rhs=Wv_s[k][:],
                                 start=(k == 0), stop=(k == KC - 1))
            nc.vector.tensor_copy(out=v_s[:, j, :], in_=psV[:, :D])
        for h in range(NH):
            dst = bass.AP(tensor=vtab.tensor, offset=(h * LPx + sup) * HD,
                          ap=[[HD, 128], [128 * HD, ntile], [1, HD]])
            src = bass.AP(tensor=v_s.tensor, offset=v_s[:].offset + h * HD,
                          ap=[v_s[:].ap[0], [D, ntile], [1, HD]])
            nc.sync.dma_start(out=dst, in_=src)

    # zero the vtab tail (expansion over-reads it; must be finite)
    ztile = consts.tile([128, 64], FP16, tag="ztile")
    nc.vector.memset(ztile[:], 0.0)
    tail = bass.AP(tensor=vtab.tensor, offset=(NH * LPx) * HD,
                   ap=[[64, 128], [1, 64]])
    nc.sync.dma_start(out=tail, in_=bass.AP(
        tensor=ztile.tensor, offset=ztile[:].offset,
        ap=[ztile[:].ap[0], [1, 64]]))

    # stencil expansion: vtabS[h, i=(y,x)] = 2x2 block [(y+a, x+b)] as 4*HD
    for h in range(NH):
        for li, (Hl, Wl) in enumerate(shapes):
            HW = Hl * Wl
            for a in range(2):
                for b in range(2):
                    sdst = bass.AP(
                        tensor=vtabS.tensor,
                        offset=(h * Lx + lvl_start[li]) * 4 * HD + (a * 2 + b) * HD,
                        ap=[[4 * HD, HW], [1, HD]])
                    ssrc = bass.AP(
                        tensor=vtab.tensor,
                        offset=(h * LPx + lvl_start[li] + a * Wl + b) * HD,
                        ap=[[HD, HW], [1, HD]])
                    nc.sync.dma_start(out=sdst, in_=ssrc)

    if dbg:
        nc.sync.dma_start(out=dbg["d_vtab"][:, :], in_=vtab[0:2048, :])

    # ================= Phase M: main =================
    def compute_fm(fm0, T):
        nb = T // 128
        halves = [(n0, min(512, T - n0)) for n0 in range(0, T, 512)]

        def ct(tag, dt=FP32, bufs=None):
            if bufs:
                return chain.tile([128, FMT], dt, tag=tag, name=tag, bufs=bufs)
            return chain.tile([128, FMT], dt, tag=tag, name=tag)

        q_t = []
        qFM = fmp.tile([128, KC, FMT], FP16, tag="qFM")
        for j in range(nb):
            t0 = fm0 + j * 128
            qt = fmp.tile([128, D], FP32, tag="q_t", name="q_t", bufs=3)
            pt = vtmp.tile([128, D], FP32, tag="pos_t")
            nc.sync.dma_start(out=qt[:], in_=io["srcq"][t0:t0 + 128, :])
            nc.sync.dma_start(out=pt[:], in_=io["posq"][t0:t0 + 128, :])
            nc.vector.tensor_add(out=qt[:], in0=qt[:], in1=pt[:])
            q_t.append(qt)
            for k in range(KC):
                tp128(qt[:, k * 128:(k + 1) * 128], qFM[:, k, j * 128:(j + 1) * 128])

        # offsets FM (boffP includes the -0.5)
        off_f = [ct("s0"), ct("s1")]
        for xy in range(2):
            for (n0, nn) in halves:
                ps = psMM.tile([128, 512], FP32, tag="mm")
                for k in range(KC):
                    nc.tensor.matmul(
                        ps[:, :nn], lhsT=Woff_s[k][:, xy * 128:(xy + 1) * 128],
                        rhs=qFM[:, k, n0:n0 + nn], start=(k == 0), stop=(k == KC - 1))
                nc.vector.tensor_scalar(off_f[xy][:, n0:n0 + nn], ps[:, :nn],
                                        boffP[:, xy:xy + 1], None, OP.add)

        # attention FM + grouped softmax (no max-sub: logits are O(1))
        expt = ct("s2")
        a_fm = ct("s3", FP16)
        r8 = chain.tile([NH, FMT], FP32, tag="r8")
        for (n0, nn) in halves:
            ps = psMM.tile([128, 512], FP32, tag="mm")
            for k in range(KC):
                nc.tensor.matmul(ps[:, :nn], lhsT=Wa_s[k][:],
                                 rhs=qFM[:, k, n0:n0 + nn],
                                 start=(k == 0), stop=(k == KC - 1))
            nc.scalar.activation(out=expt[:, n0:n0 + nn], in_=ps[:, :nn],
                                 func=AF.Exp, bias=ba_f[:, 0:1], scale=1.0)
            ps8 = psS.tile([NH, 512], FP32, tag="s8")
            nc.tensor.matmul(ps8[:, :nn], lhsT=hsum[:], rhs=expt[:, n0:n0 + nn],
                             start=True, stop=True)
            nc.vector.reciprocal(out=r8[:, n0:n0 + nn], in_=ps8[:, :nn])
            psr = psMM.tile([128, 512], FP32, tag="mm")
            nc.tensor.matmul(psr[:, :nn], lhsT=hsumT[:NH, :], rhs=r8[:, n0:n0 + nn],
                             start=True, stop=True)
            nc.vector.tensor_tensor(out=a_fm[:, n0:n0 + nn],
                                    in0=expt[:, n0:n0 + nn], in1=psr[:, :nn],
                                    op=OP.mult)

        # reference points FM (replicated over heads/points)
        ref_f = []
        for xy, nm in ((0, "refx_fm"), (1, "refy_fm")):
            rt = ct(f"s{4 + xy}")
            nc.sync.dma_start(out=rt[:, :T], in_=io[nm][:, fm0:fm0 + T])
            ref_f.append(rt)

        # ---- FM coordinate / weight / index chain ----
        # Stencil gather: one idx per sample = base + yc*W + xc where
        # xc = clamp(floor(x), 0, W-1), yc likewise. Stencil row holds the
        # 2x2 block [(yc..yc+1) x (xc..xc+1)]; slot weights zero out any
        # invalid corner. Floor is cast-rounding-mode independent:
        # r = float(int(v)); r -= (r > v).
        def axis_chain(ref_t, off_t, scale_ap, lim1_ap, lim2_ap,
                       s6, s7, s8, s9, s10, s11, o_c, o_we, o_wo):
            x = off_t
            nc.vector.scalar_tensor_tensor(out=x[:, :T], in0=ref_t[:, :T],
                                           scalar=scale_ap, in1=off_t[:, :T],
                                           op0=OP.mult, op1=OP.add)
            xcl = ref_t
            nc.vector.tensor_scalar(xcl[:, :T], x[:, :T], -1.0, scale_ap,
                                    OP.max, OP.min)
            xp1 = off_t
            nc.vector.tensor_scalar(xp1[:, :T], xcl[:, :T], 1.0, None, OP.add)
            xi = chain.tile([128, FMT], I32, tag="si", name="si")
            nc.vector.tensor_copy(out=xi[:, :T], in_=xp1[:, :T])
            rf = s6
            nc.vector.tensor_copy(out=rf[:, :T], in_=xi[:, :T])
            g = s7
            nc.vector.tensor_tensor(out=g[:, :T], in0=rf[:, :T], in1=xp1[:, :T],
                                    op=OP.is_gt)
            nc.vector.tensor_tensor(out=rf[:, :T], in0=rf[:, :T], in1=g[:, :T],
                                    op=OP.subtract)      # rf = floor(xp1)
            fx = s7
            nc.vector.tensor_tensor(out=fx[:, :T], in0=xp1[:, :T], in1=rf[:, :T],
                                    op=OP.subtract)
            x0 = off_t
            nc.vector.tensor_scalar(x0[:, :T], rf[:, :T], 1.0, None, OP.subtract)
            fxc = s8
            nc.scalar.activation(out=fxc[:, :T], in_=fx[:, :T],
                                 func=AF.Copy, bias=1.0, scale=-1.0)
            nc.vector.tensor_scalar(o_c[:, :T], x0[:, :T], 0.0, lim1_ap,
                                    OP.max, OP.min)      # xc
            modd = s11
            nc.vector.tensor_scalar(modd[:, :T], x0[:, :T], lim2_ap, None,
                                    OP.is_le)            # x0 <= W-2
            d = s6
            nc.vector.tensor_tensor(out=d[:, :T], in0=x0[:, :T], in1=o_c[:, :T],
                                    op=OP.subtract)
            a0 = s9
            nc.vector.tensor_scalar(a0[:, :T], d[:, :T], 0.0, None, OP.is_equal)
            am = s10
            nc.vector.tensor_scalar(am[:, :T], d[:, :T], -1.0, None, OP.is_equal)
            t1, t2 = s6, off_t
            nc.vector.tensor_tensor(out=t1[:, :T], in0=fxc[:, :T], in1=a0[:, :T], op=OP.mult)
            nc.vector.tensor_tensor(out=t2[:, :T], in0=fx[:, :T], in1=am[:, :T], op=OP.mult)
            nc.vector.tensor_add(out=o_we[:, :T], in0=t1[:, :T], in1=t2[:, :T])
            nc.vector.tensor_tensor(out=t1[:, :T], in0=fx[:, :T], in1=a0[:, :T], op=OP.mult)
            nc.vector.tensor_tensor(out=o_wo[:, :T], in0=t1[:, :T], in1=modd[:, :T], op=OP.mult)

        s6, s7, s8, s9 = ct("s6"), ct("s7"), ct("s8"), ct("s9")
        s10, s11 = ct("s10"), ct("s11")
        xc, yc = ct("xc"), ct("yc")
        wxe, wxo = ct("wxe", FP16), ct("wxo", FP16)
        wye_r, wyo_r = ct("wye_r", FP16), ct("wyo_r", FP16)
        axis_chain(ref_f[0], off_f[0], Wf, Wm1, Wm2,
                   s6, s7, s8, s9, s10, s11, xc, wxe, wxo)
        axis_chain(ref_f[1], off_f[1], Hf, Hm1, Hm2,
                   s6, s7, s8, s9, s10, s11, yc, wye_r, wyo_r)
        wye, wyo = ct("wye", FP16), ct("wyo", FP16)
        nc.vector.tensor_tensor(out=wye[:, :T], in0=wye_r[:, :T], in1=a_fm[:, :T], op=OP.mult)
        nc.vector.tensor_tensor(out=wyo[:, :T], in0=wyo_r[:, :T], in1=a_fm[:, :T], op=OP.mult)

        corners = []
        for i, (wy, wx) in enumerate(((wye, wxe), (wye, wxo), (wyo, wxe), (wyo, wxo))):
            c = ct(f"c{i}", bufs=2)
            nc.vector.tensor_tensor(out=c[:, :T], in0=wy[:, :T], in1=wx[:, :T], op=OP.mult)
            corners.append(c)

        ix = ct("ix", bufs=2)
        nc.vector.scalar_tensor_tensor(out=ix[:, :T], in0=yc[:, :T], scalar=Wf,
                                       in1=xc[:, :T], op0=OP.mult, op1=OP.add)
        nc.vector.tensor_scalar(ix[:, :T], ix[:, :T], basef, None, OP.add)

        if dbg and fm0 == 0:
            nc.sync.dma_start(out=dbg["d_afm"][:, :T], in_=a_fm[:, :T])
            nc.sync.dma_start(out=dbg["d_it"][0, :, :T], in_=ix[:, :T])

        def run_blocks():
            _run_blocks(fm0, nb, ix, corners)

        return run_blocks

    def _run_blocks(fm0, nb, ix, corners):
        for j in range(nb):
            t0 = fm0 + j * 128
            sl = slice(j * 128, (j + 1) * 128)
            # wrapped+replicated idx tile: desc i (= lp*128 + t) must sit at
            # [i%16, i//16] in each 16-partition group. Achieved by PE-
            # transposing a free-replicated view of ix: psum[r*16+q, f] =
            # ix[f, 16*jj+q]; ACT-copy (cast->int16) into cols lp*8+jj.
            idxw = blk.tile([128, NH, 16, 8], mybir.dt.int16, tag="idxw", bufs=3)
            for jj in range(8):
                rep = bass.AP(tensor=ix.tensor,
                              offset=ix[:].offset + j * 128 + jj * 16,
                              ap=[ix[:].ap[0], [0, 8], [1, 16]])
                rep8 = blk.tile([128, 128], FP32, tag="rep8")
                nc.scalar.copy(out=rep8[:], in_=rep)
                ps = psTP.tile([128, 128], FP32, tag="tp", name="tp")
                nc.tensor.transpose(out=ps[:], in_=rep8[:], identity=ident[:])
                nc.scalar.copy(out=idxw[:, :, :, jj], in_=ps[:])
            W4 = blk.tile([128, 128, 4], FP16, tag="W4")
            for ci, c in enumerate(corners):
                tp128(c[:, sl], W4[:, :, ci])

            if dbg and fm0 == 0:
                nc.sync.dma_start(out=dbg["d_w4"][j, :, :],
                                  in_=W4[:].rearrange("p a b -> p (a b)"))
            ao = blk.tile([128, D], FP32, tag="ao")
            for h in range(NH):
                G2 = gpool.tile([128, 16, 128], FP16, tag="G2", bufs=5)
                nc.gpsimd.dma_gather(
                    out_ap=G2[:],
                    in_ap=vtabS[h * Lx:(h + 1) * Lx, :],
                    idxs_ap=idxw[:, h, :, :],
                    num_idxs=2048, num_idxs_reg=2048, elem_size=4 * HD,
                    single_packet=False)
                if dbg and fm0 == 0 and h == 0:
                    nc.sync.dma_start(out=dbg["d_g2"][j, :, :2048],
                                      in_=G2[:].rearrange("p a b -> p (a b)"))
                w4h = bass.AP(tensor=W4.tensor, offset=W4[:].offset + h * 64,
                              ap=[W4[:].ap[0], [1, 64], [0, 32]])
                GW = gpool.tile([128, 64, 32], FP16, tag="GW")
                gwf = bass.AP(tensor=GW.tensor, offset=GW[:].offset,
                              ap=[GW[:].ap[0], [1, 2048]])
                nc.vector.tensor_tensor(
                    out=gwf, in0=G2[:].rearrange("p a b -> p (a b)"),
                    in1=w4h, op=OP.mult)
                nc.vector.tensor_reduce(
                    out=ao[:, h * HD:(h + 1) * HD],
                    in_=GW[:].transpose([0, 2, 1]), axis=AX.X, op=OP.add)

            aoFM = blk.tile([128, KC, 128], FP16, tag="aoFM")
            for k in range(KC):
                tp128(ao[:, k * 128:(k + 1) * 128], aoFM[:, k, :])
            psO = psMM.tile([128, 512], FP32, tag="mm")
            for k in range(KC):
                nc.tensor.matmul(psO[:, :D], lhsT=aoFM[:, k, :], rhs=Wo_s[k][:],
                                 start=(k == 0), stop=(k == KC - 1))
            s_t = vtmp.tile([128, D], FP32, tag="s_res")
            nc.sync.dma_start(out=s_t[:], in_=io["srcq"][t0:t0 + 128, :])
            x1 = blk.tile([128, D], FP32, tag="x1")
            nc.vector.scalar_tensor_tensor(out=x1[:], in0=psO[:, :D], scalar=1.0,
                                           in1=s_t[:], op0=OP.mult, op1=OP.add)
            if dbg and fm0 == 0:
                nc.sync.dma_start(out=dbg["d_ao"][j, :, :], in_=ao[:])
                nc.sync.dma_start(out=dbg["d_x1"][j, :, :], in_=x1[:])
            _layernorm(nc, blk, x1, g1_rep, be1_rep, "1", eps_t)
            nc.sync.dma_start(out=x1n_dram[t0:t0 + 128, :], in_=x1[:])

    pending = None
    for (_fm0, _T) in _fm_blocks(QPx):
        nxt = compute_fm(_fm0, _T)
        if pending is not None:
            pending()
        pending = nxt
    if pending is not None:
        pending()

    # ================= Phase F: FFN =================
    for sb0 in range(0, QPx, 256):
        ST = min(256, QPx - sb0)
        nt = ST // 128
        x_t = []
        xFM = fmp.tile([128, KC, 512], FP16, tag="xFM")
        for j in range(nt):
            t0 = sb0 + j * 128
            xt = fmp.tile([128, D], FP32, tag=f"x_t{j}")
            nc.sync.dma_start(out=xt[:], in_=x1n_dram[t0:t0 + 128, :])
            x_t.append(xt)
            for k in range(KC):
                tp128(xt[:, k * 128:(k + 1) * 128], xFM[:, k, j * 128:(j + 1) * 128])
        h1FM = fmp.tile([128, MF, 256], FP16, tag="h1FM")
        for m in range(MF):
            psH = psMM.tile([128, 512], FP32, tag="mm")
            for k in range(KC):
                nc.tensor.matmul(psH[:, :ST], lhsT=W1_s[k][:, m * 128:(m + 1) * 128],
                                 rhs=xFM[:, k, :ST], start=(k == 0), stop=(k == KC - 1))
            nc.scalar.activation(out=h1FM[:, m, :ST], in_=psH[:, :ST],
                                 func=AF.Relu, bias=b1c[:, m:m + 1], scale=1.0)
        for j in range(nt):
            t0 = sb0 + j * 128
            psH2 = psMM.tile([128, 512], FP32, tag="mm")
            for k in range(MF):
                nc.tensor.matmul(psH2[:, :D], lhsT=h1FM[:, k, j * 128:(j + 1) * 128],
                                 rhs=W2_s[k][:], start=(k == 0), stop=(k == MF - 1))
            x2 = blk.tile([128, D], FP32, tag="x2")
            nc.vector.scalar_tensor_tensor(out=x2[:], in0=psH2[:, :D], scalar=1.0,
                                           in1=x_t[j][:], op0=OP.mult, op1=OP.add)
            _layernorm(nc, blk, x2, g2_rep, be2_rep, "2", eps_t)
            nc.sync.dma_start(out=out_ap[t0:t0 + 128, :], in_=x2[:])


def _layernorm(nc, pool, x, g_rep, be_rep, tag, eps_t=None):
    stats = pool.tile([128, 6], FP32, tag=f"st{tag}")
    nc.vector.bn_stats(out=stats[:], in_=x[:])
    mv = pool.tile([128, 2], FP32, tag=f"mv{tag}")
    nc.vector.bn_aggr(out=mv[:], in_=stats[:])
    sd = pool.tile([128, 1], FP32, tag=f"sd{tag}")
    nc.scalar.activation(out=sd[:], in_=mv[:, 1:2], func=AF.Sqrt,
                         bias=eps_t[:, 0:1], scale=1.0)
    rstd = pool.tile([128, 1], FP32, tag=f"rs{tag}")
    nc.vector.reciprocal(out=rstd[:], in_=sd[:])
    nc.vector.tensor_scalar(x[:], x[:], mv[:, 0:1], rstd[:, 0:1],
                            OP.subtract, OP.mult)
    nc.vector.tensor_tensor(out=x[:], in0=x[:], in1=g_rep[:], op=OP.mult)
    nc.vector.tensor_tensor(out=x[:], in0=x[:], in1=be_rep[:], op=OP.add)


# ================= host side =================

def _perm_off():
    return np.concatenate([np.arange(0, D, 2), np.arange(1, D, 2)])


def make_host_consts(shapes, inputs):
    lvl = np.cumsum([0] + [h * w for h, w in shapes])[:-1]
    _, _, _, _, LPx = _plan(shapes)
    f_h = np.arange(128) // 16
    f_l = (np.arange(128) // NP) % NL
    Wl = np.array([w for h, w in shapes], np.float32)
    Hl = np.array([h for h, w in shapes], np.float32)
    fconst = np.zeros((128, 7), np.float32)
    fconst[:, 0] = Wl[f_l]
    fconst[:, 1] = Hl[f_l]
    fconst[:, 2] = Wl[f_l] - 1.0
    fconst[:, 3] = Hl[f_l] - 1.0
    fconst[:, 4] = np.asarray(lvl)[f_l]
    fconst[:, 5] = Wl[f_l] - 2.0
    fconst[:, 6] = Hl[f_l] - 2.0
    perm = _perm_off()
    WoffP = np.asarray(inputs["Woff"], np.float32)[:, perm]
    boffP = (np.asarray(inputs["boff"], np.float32)[perm] - 0.5).reshape(2, 128).T.copy()
    hsum = np.zeros((128, NH), np.float32)
    hsum[np.arange(128), f_h] = 1.0
    hsumT = np.ascontiguousarray(hsum.T)
    b1c = np.asarray(inputs["b1"], np.float32).reshape(DF // 128, 128).T.copy()
    ba_f = np.asarray(inputs["ba"], np.float32).reshape(128, 1).copy()
    pred = np.asarray(inputs["dataset_group_pred"], np.float32)
    lns = {}
    for b in range(pred.shape[0]):
        lns[b] = dict(
            g1=np.ascontiguousarray(pred[b] @ np.asarray(inputs["gw1"], np.float32)),
            be1=np.ascontiguousarray(pred[b] @ np.asarray(inputs["gb1"], np.float32)),
            g2=np.ascontiguousarray(pred[b] @ np.asarray(inputs["gw2"], np.float32)),
            be2=np.ascontiguousarray(pred[b] @ np.asarray(inputs["gb2"], np.float32)),
        )
    return dict(fconst=fconst, WoffP=WoffP, boffP=boffP, hsum=hsum,
                hsumT=hsumT, b1c=b1c, ba_f=ba_f, lns=lns)


def make_core_inputs(core, inputs, shapes=SHAPES, hc=None):
    Lx, _, Qx, QPx, LPx = _plan(shapes)
    if hc is None:
        hc = make_host_consts(shapes, inputs)
    b, s = core // NSHARD, core % NSHARD
    src = np.asarray(inputs["src"], np.float32)
    pos = np.asarray(inputs["pos"], np.float32)
    ref = np.asarray(inputs["reference_points"], np.float32)

    def padQ(a, extra):
        out = np.zeros((QPx,) + extra, np.float32)
        out[:Qx] = a[b, s * Qx:(s + 1) * Qx]
        return out

    src_full = np.zeros((LPx, D), np.float32)
    src_full[:Lx] = src[b]
    refq = padQ(ref, (NL, 2))           # [QP, NL, 2]
    f_l = (np.arange(128) // NP) % NL
    refx_fm = np.ascontiguousarray(refq[:, f_l, 0].T)  # [128, QP]
    refy_fm = np.ascontiguousarray(refq[:, f_l, 1].T)
    f16 = lambda k: np.asarray(inputs[k], np.float32).astype(np.float16)
    return dict(
        src_full=src_full,
        srcq=padQ(src, (D,)),
        posq=padQ(pos, (D,)),
        refx_fm=refx_fm, refy_fm=refy_fm,
        Wvh=f16("Wv"),
        WoffPh=hc["WoffP"].astype(np.float16),
        Wah=f16("Wa"),
        Woh=f16("Wo"),
        W1h=f16("W1"),
        W2h=f16("W2"),
        b1c=hc["b1c"], ba_f=hc["ba_f"], boffP=hc["boffP"],
        fconst=hc["fconst"],
        hsum=hc["hsum"], hsumT=hc["hsumT"],
        g1=hc["lns"][b]["g1"], be1=hc["lns"][b]["be1"],
        g2=hc["lns"][b]["g2"], be2=hc["lns"][b]["be2"],
    )


_PROGRAM = None


def _get_program():
    global _PROGRAM
    if _PROGRAM is None:
        _PROGRAM = build_program()
    return _PROGRAM


def _ensure_ntff_hook():
    """Shim antenv.axon_hooks (absent in this image) and register the
    ctypes NTFF profile hook against the injected libaxon so."""
    import types
    if "antenv.axon_hooks" in sys.modules:
        return
    mod = types.ModuleType("antenv.axon_hooks")
    mod._hook = None
    mod.set_axon_ntff_profile_hook = lambda h: setattr(mod, "_hook", h)
    mod.get_axon_ntff_profile_hook = lambda: mod._hook
    sys.modules["antenv.axon_hooks"] = mod
    try:
        from trn_agent_boot.trn_boot import _ntff_profile_via_ctypes
        mod._hook = _ntff_profile_via_ctypes("/opt/axon/libaxon_pjrt.so")
    except Exception as e:
        print(f"ntff hook registration failed: {e}")


def run(inputs, trace=False):
    if trace:
        _ensure_ntff_hook()
    from concourse.bass_utils import run_bass_kernel_spmd
    nc = _get_program()
    hc = make_host_consts(SHAPES, inputs)
    in_maps = [make_core_inputs(c, inputs, hc=hc) for c in range(NCORES)]
    res = run_bass_kernel_spmd(nc, in_maps, core_ids=list(range(NCORES)),
                               trace=trace)
    out = np.zeros((B, L, D), np.float32)
    for c in range(NCORES):
        b, s = c // NSHARD, c % NSHARD
        out[b, s * Q:(s + 1) * Q] = res.results[c]["out"][:Q]
    return out, res


def kernel(**inputs):
    out, _ = run(inputs, trace=False)
    return out



# revision 15
# speedup vs baseline: 1.0170x; 1.0170x over previous
"""Deformable transformer encoder layer on 8 TRN2 NeuronCores.

Sharding: core c -> (batch b=c//4, quarter s=c%4) of the 19560 query tokens
(padded to 4992 = 39*128). Each core computes the full-batch value table
(redundantly, avoids collectives) and writes the 2x2-stencil-expanded table
vtabS[h, entry=(y,x)] = [v(y,x), v(y,x+1), v(y+1,x), v(y+1,x+1)] (256B fp16
entries) DIRECTLY from SBUF with 4 shifted slot writes (no DRAM->DRAM
expansion pass). Entry coords are clamped to [0, W-2]x[0, H-2] so every
referenced entry has all 4 slot pixels inside its level; slot weights carry
the d = x0-xc in {-1,0,+1} selection plus border-validity zeroing. Bilinear
sampling gathers one 256B stencil entry per (token, head, level, point) with
the Ant indirect DMA. The FFN runs interleaved per 256 tokens straight from
SBUF (no DRAM roundtrip). Matmuls run fp16 x fp16 -> fp32 PSUM; coordinates
and LN run fp32.
"""
import sys, os
sys.path.insert(0, "/opt/trn_rl_repo")

import numpy as np
from contextlib import ExitStack

import concourse.bass as bass
import concourse.tile as tile
from concourse import bacc, mybir
from concourse.masks import make_identity

FP32 = mybir.dt.float32
FP16 = mybir.dt.float16
I32 = mybir.dt.int32
AX = mybir.AxisListType
OP = mybir.AluOpType
AF = mybir.ActivationFunctionType

SHAPES = [(92, 160), (46, 80), (23, 40), (12, 20)]
NH, HD, NL, NP = 8, 32, 4, 4
D = NH * HD
DF = 4 * D
EPS = 1e-5
B = 2
NCORES = 8
NSHARD = 4
FMT = 1024  # FM-block token width
NQUEUES = 1  # SWDGE queues for gathers


def _plan(shapes):
    L = sum(h * w for h, w in shapes)
    lvl_start = np.cumsum([0] + [h * w for h, w in shapes])[:-1].tolist()
    Q = (L + NSHARD - 1) // NSHARD
    QP = ((Q + 127) // 128) * 128
    LP = ((L + 127) // 128) * 128
    return L, lvl_start, Q, QP, LP


L, LVL_START, Q, QP, LP = _plan(SHAPES)    # 19560, ..., 4890, 4992, 19584


def _fm_blocks(T, w=FMT):
    out, t0 = [], 0
    while t0 < T:
        out.append((t0, min(w, T - t0)))
        t0 += w
    return out


def build_program(shapes=SHAPES):
    Lx, lvl_start, Qx, QPx, LPx = _plan(shapes)
    NPIX = NH * LPx

    nc = bacc.Bacc("TRN2", target_bir_lowering=False, debug=False,
                   enable_asserts=False, num_devices=1,
                   num_swdge_queues=NQUEUES)

    def din(name, shape, dt=FP32):
        return nc.dram_tensor(name, list(shape), dt, kind="ExternalInput").ap()

    io = {
        "src_full": din("src_full", [LPx, D], FP16),
        "srcq": din("srcq", [QPx, D]),
        "posq": din("posq", [QPx, D]),
        "refx_fm": din("refx_fm", [128, QPx]),
        "refy_fm": din("refy_fm", [128, QPx]),
        "Wvh": din("Wvh", [D, D], FP16),
        "WoffPh": din("WoffPh", [D, D], FP16),
        "Wah": din("Wah", [D, 128], FP16),
        "Woh": din("Woh", [D, D], FP16),
        "W1h": din("W1h", [D, DF], FP16),
        "W2h": din("W2h", [DF, D], FP16),
        "b1c": din("b1c", [128, DF // 128]),
        "ba_f": din("ba_f", [128, 1]),
        "boffP": din("boffP", [128, 2]),
        "fconst": din("fconst", [128, 7]),
        "hsum": din("hsum", [128, NH]),
        "hsumT": din("hsumT", [NH, 128]),
        "g1": din("g1", [D]), "be1": din("be1", [D]),
        "g2": din("g2", [D]), "be2": din("be2", [D]),
    }
    out_ap = nc.dram_tensor("out", [QPx, D], FP32, kind="ExternalOutput").ap()
    dbg = {}
    if os.environ.get("KDBG"):
        dbg = {
            "d_w4": nc.dram_tensor("d_w4", [8, 128, 512], FP16, kind="ExternalOutput").ap(),
            "d_g2": nc.dram_tensor("d_g2", [8, 128, 4096], FP16, kind="ExternalOutput").ap(),
            "d_ao": nc.dram_tensor("d_ao", [8, 128, 256], FP32, kind="ExternalOutput").ap(),
            "d_x1": nc.dram_tensor("d_x1", [8, 128, 256], FP32, kind="ExternalOutput").ap(),
            "d_afm": nc.dram_tensor("d_afm", [128, 1024], FP16, kind="ExternalOutput").ap(),
            "d_it": nc.dram_tensor("d_it", [2, 128, 1024], FP32, kind="ExternalOutput").ap(),
            "d_vtabS": nc.dram_tensor("d_vtabS", [2048, 128], FP16, kind="ExternalOutput").ap(),
        }
    vtabS = nc.dram_tensor("vtabS", [NH * Lx, 4 * HD], FP16).ap()

    with tile.TileContext(nc) as tc:
        with ExitStack() as ctx:
            _body(ctx, tc, io, out_ap, vtabS,
                  shapes, lvl_start, Lx, QPx, LPx, dbg)
    nc.compile()
    return nc


def _body(ctx, tc, io, out_ap, vtabS, shapes, lvl_start, Lx, QPx, LPx, dbg={}):
    nc = tc.nc
    KC = D // 128
    MF = DF // 128

    consts = ctx.enter_context(tc.tile_pool(name="consts", bufs=1))
    vpool = ctx.enter_context(tc.tile_pool(name="vpool", bufs=2))
    vtmp = ctx.enter_context(tc.tile_pool(name="vtmp", bufs=2))
    fmp = ctx.enter_context(tc.tile_pool(name="fmp", bufs=1))
    chain = ctx.enter_context(tc.tile_pool(name="chain", bufs=1))
    blk = ctx.enter_context(tc.tile_pool(name="blk", bufs=2))
    gpool = ctx.enter_context(tc.tile_pool(name="gpool", bufs=2))
    psTP = ctx.enter_context(tc.tile_pool(name="psTP", bufs=2, space="PSUM"))
    psMM = ctx.enter_context(tc.tile_pool(name="psMM", bufs=3, space="PSUM"))
    psS = ctx.enter_context(tc.tile_pool(name="psS", bufs=1, space="PSUM"))

    # ---- constants ----
    ident = consts.tile([128, 128], FP32)
    make_identity(nc, ident[:])
    identh = consts.tile([128, 128], FP16, tag="identh")
    make_identity(nc, identh[:])

    def wtiles(name, ap, ncol):
        ts = []
        for k in range(ap.shape[0] // 128):
            t = consts.tile([128, ncol], FP16, tag=f"{name}{k}")
            nc.sync.dma_start(out=t[:], in_=ap[k * 128:(k + 1) * 128, :])
            ts.append(t)
        return ts

    Wv_s = wtiles("Wv", io["Wvh"], D)
    Woff_s = wtiles("Woff", io["WoffPh"], D)
    Wa_s = wtiles("Wa", io["Wah"], 128)
    Wo_s = wtiles("Wo", io["Woh"], D)
    W1_s = wtiles("W1", io["W1h"], DF)
    W2_s = wtiles("W2", io["W2h"], D)

    def rep_tile(name, ap):
        t = consts.tile([128, D], FP32, tag=name)
        bap = bass.AP(tensor=ap.tensor, offset=ap.offset, ap=[[0, 128], [1, D]])
        nc.gpsimd.dma_start(out=t[:], in_=bap)
        return t

    g1_rep = rep_tile("g1_rep", io["g1"])
    be1_rep = rep_tile("be1_rep", io["be1"])
    g2_rep = rep_tile("g2_rep", io["g2"])
    be2_rep = rep_tile("be2_rep", io["be2"])

    def load(name, ap, dt=FP32):
        t = consts.tile(list(ap.shape), dt, tag=name)
        nc.sync.dma_start(out=t[tuple(slice(0, s) for s in ap.shape)],
                          in_=ap[tuple(slice(None) for _ in ap.shape)])
        return t

    eps_t = consts.tile([128, 1], FP32, tag="eps_t")
    nc.vector.memset(eps_t[:], EPS)
    b1c = load("b1c", io["b1c"])
    ba_f = load("ba_f", io["ba_f"])
    boffP = load("boffP", io["boffP"])
    fconst = load("fconst", io["fconst"])
    hsum = load("hsum", io["hsum"])
    hsumT = load("hsumT", io["hsumT"])
    Wf, Hf = fconst[:, 0:1], fconst[:, 1:2]
    Wm1, Hm1 = fconst[:, 2:3], fconst[:, 3:4]
    basef = fconst[:, 4:5]
    Wm2, Hm2 = fconst[:, 5:6], fconst[:, 6:7]

    def tp128(src_ap, dst_ap, engine=None):
        assert src_ap.dtype == FP32
        ps = psTP.tile([128, 128], FP32, tag="tp", name="tp")
        nc.tensor.transpose(out=ps[:], in_=src_ap, identity=ident[:])
        if engine is nc.vector:
            nc.vector.tensor_copy(out=dst_ap, in_=ps[:])
        else:
            nc.scalar.copy(out=dst_ap, in_=ps[:])

    # ================= Phase V: value table =================
    # vtabS entry e=(y,x) of (head, level) holds the 2x2 pixel block as 4
    # slots [v(y,x), v(y,x+1), v(y+1,x), v(y+1,x+1)], written directly from
    # the SBUF value tiles as 4 shifted strided writes (slot (a,b) of entry
    # e is pixel e + a*W + b). Gathers only touch entries with y<=H-2,
    # x<=W-2, whose 4 slot pixels all lie inside the level, so every
    # referenced byte is written; unreferenced entries may hold garbage.
    def tp128h(src_ap, dst_ap):
        ps = psTP.tile([128, 128], FP16, tag="tph", name="tph")
        nc.tensor.transpose(out=ps[:], in_=src_ap, identity=identh[:])
        nc.scalar.copy(out=dst_ap, in_=ps[:])

    eng_rr = [nc.sync, nc.scalar]
    for sup in range(0, LPx, 1024):
        ntile = min(8, (LPx - sup) // 128)
        sup1 = sup + ntile * 128
        v_s = vpool.tile([128, 8, D], FP16, tag="v_s")
        for j in range(ntile):
            t0 = sup + j * 128
            s_t = vtmp.tile([128, D], FP16, tag="s_t")
            nc.sync.dma_start(out=s_t[:], in_=io["src_full"][t0:t0 + 128, :])
            sFM = vtmp.tile([128, KC, 128], FP16, tag="sFM")
            for k in range(KC):
                tp128h(s_t[:, k * 128:(k + 1) * 128], sFM[:, k, :])
            psV = psMM.tile([128, 512], FP32, tag="mm")
            for k in range(KC):
                nc.tensor.matmul(psV[:, :D], lhsT=sFM[:, k, :], rhs=Wv_s[k][:],
                                 start=(k == 0), stop=(k == KC - 1))
            nc.vector.tensor_copy(out=v_s[:, j, :], in_=psV[:, :D])
        ei = 0
        for li, (Hl, Wl) in enumerate(shapes):
            st = lvl_start[li]
            en = st + Hl * Wl
            for a in range(2):
                for b in range(2):
                    p0 = max(sup, st + a * Wl + b)
                    p1 = min(sup1, en)
                    if p1 <= p0:
                        continue
                    pieces = []
                    rel0, rel1 = p0 - sup, p1 - sup
                    if rel0 % 128:
                        c = min(128 - rel0 % 128, rel1 - rel0)
                        pieces.append((rel0, c, 1))
                        rel0 += c
                    nfull = (rel1 - rel0) // 128
                    if nfull:
                        pieces.append((rel0, 128, nfull))
                        rel0 += nfull * 128
                    if rel1 > rel0:
                        pieces.append((rel0, rel1 - rel0, 1))
                    for (rel, cnt, nj) in pieces:
                        j0, pp = rel // 128, rel % 128
                        e0 = sup + rel - st - a * Wl - b
                        for h in range(NH):
                            dst = bass.AP(
                                tensor=vtabS.tensor,
                                offset=((h * Lx + st + e0) * 4
                                        + (a * 2 + b)) * HD,
                                ap=[[4 * HD, cnt], [128 * 4 * HD, nj],
                                    [1, HD]])
                            src = v_s[pp:pp + cnt, j0:j0 + nj,
                                      h * HD:(h + 1) * HD]
                            eng_rr[ei % 2].dma_start(out=dst, in_=src)
                            ei += 1

    if dbg:
        nc.sync.dma_start(out=dbg["d_vtabS"][:, :], in_=vtabS[0:2048, :])

    # ================= Phase M: main =================
    def compute_fm(fm0, T):
        nb = T // 128
        halves = [(n0, min(512, T - n0)) for n0 in range(0, T, 512)]

        def ct(tag, dt=FP32, bufs=None):
            if bufs:
                return chain.tile([128, FMT], dt, tag=tag, name=tag, bufs=bufs)
            return chain.tile([128, FMT], dt, tag=tag, name=tag)

        q_t = []
        qFM = fmp.tile([128, KC, FMT], FP16, tag="qFM")
        for j in range(nb):
            t0 = fm0 + j * 128
            qt = fmp.tile([128, D], FP32, tag="q_t", name="q_t", bufs=3)
            pt = vtmp.tile([128, D], FP32, tag="pos_t")
            nc.sync.dma_start(out=qt[:], in_=io["srcq"][t0:t0 + 128, :])
            nc.sync.dma_start(out=pt[:], in_=io["posq"][t0:t0 + 128, :])
            nc.vector.tensor_add(out=qt[:], in0=qt[:], in1=pt[:])
            q_t.append(qt)
            for k in range(KC):
                tp128(qt[:, k * 128:(k + 1) * 128], qFM[:, k, j * 128:(j + 1) * 128])

        # offsets FM (boffP includes the -0.5)
        off_f = [ct("s0"), ct("s1")]
        for xy in range(2):
            for (n0, nn) in halves:
                ps = psMM.tile([128, 512], FP32, tag="mm")
                for k in range(KC):
                    nc.tensor.matmul(
                        ps[:, :nn], lhsT=Woff_s[k][:, xy * 128:(xy + 1) * 128],
                        rhs=qFM[:, k, n0:n0 + nn], start=(k == 0), stop=(k == KC - 1))
                nc.vector.tensor_scalar(off_f[xy][:, n0:n0 + nn], ps[:, :nn],
                                        boffP[:, xy:xy + 1], None, OP.add)

        # attention FM + grouped softmax (no max-sub: logits are O(1))
        expt = ct("s2")
        a_fm = ct("s3", FP16)
        r8 = chain.tile([NH, FMT], FP32, tag="r8")
        for (n0, nn) in halves:
            ps = psMM.tile([128, 512], FP32, tag="mm")
            for k in range(KC):
                nc.tensor.matmul(ps[:, :nn], lhsT=Wa_s[k][:],
                                 rhs=qFM[:, k, n0:n0 + nn],
                                 start=(k == 0), stop=(k == KC - 1))
            nc.scalar.activation(out=expt[:, n0:n0 + nn], in_=ps[:, :nn],
                                 func=AF.Exp, bias=ba_f[:, 0:1], scale=1.0)
            ps8 = psS.tile([NH, 512], FP32, tag="s8")
            nc.tensor.matmul(ps8[:, :nn], lhsT=hsum[:], rhs=expt[:, n0:n0 + nn],
                             start=True, stop=True)
            nc.vector.reciprocal(out=r8[:, n0:n0 + nn], in_=ps8[:, :nn])
            psr = psMM.tile([128, 512], FP32, tag="mm")
            nc.tensor.matmul(psr[:, :nn], lhsT=hsumT[:NH, :], rhs=r8[:, n0:n0 + nn],
                             start=True, stop=True)
            nc.vector.tensor_tensor(out=a_fm[:, n0:n0 + nn],
                                    in0=expt[:, n0:n0 + nn], in1=psr[:, :nn],
                                    op=OP.mult)

        # reference points FM (replicated over heads/points)
        ref_f = []
        for xy, nm in ((0, "refx_fm"), (1, "refy_fm")):
            rt = ct(f"s{4 + xy}")
            nc.sync.dma_start(out=rt[:, :T], in_=io[nm][:, fm0:fm0 + T])
            ref_f.append(rt)

        # ---- FM coordinate / weight / index chain ----
        # Stencil gather: one idx per sample = base + yc*W + xc where
        # xc = clamp(floor(x), 0, W-2), yc likewise, so the 2x2 entry block
        # always lies inside the level. d = floor(x) - xc in {-1, 0, +1}
        # selects which slot holds pixel x0 / x0+1; out-of-image corners get
        # weight 0 (d==+2 when x0 = W matches nothing). Floor is cast-
        # rounding-mode independent: r = float(int(v)); r -= (r > v).
        def axis_chain(ref_t, off_t, scale_ap, lim2_ap,
                       s6, s7, s8, s9, s10, s11, o_c, o_we, o_wo):
            x = off_t
            nc.vector.scalar_tensor_tensor(out=x[:, :T], in0=ref_t[:, :T],
                                           scalar=scale_ap, in1=off_t[:, :T],
                                           op0=OP.mult, op1=OP.add)
            xcl = ref_t
            nc.vector.tensor_scalar(xcl[:, :T], x[:, :T], -1.0, scale_ap,
                                    OP.max, OP.min)
            xp1 = off_t
            nc.vector.tensor_scalar(xp1[:, :T], xcl[:, :T], 1.0, None, OP.add)
            xi = chain.tile([128, FMT], I32, tag="si", name="si")
            nc.vector.tensor_copy(out=xi[:, :T], in_=xp1[:, :T])
            rf = s6
            nc.vector.tensor_copy(out=rf[:, :T], in_=xi[:, :T])
            g = s7
            nc.vector.tensor_tensor(out=g[:, :T], in0=rf[:, :T], in1=xp1[:, :T],
                                    op=OP.is_gt)
            nc.vector.tensor_tensor(out=rf[:, :T], in0=rf[:, :T], in1=g[:, :T],
                                    op=OP.subtract)      # rf = floor(xp1)
            fx = s7
            nc.vector.tensor_tensor(out=fx[:, :T], in0=xp1[:, :T], in1=rf[:, :T],
                                    op=OP.subtract)
            x0 = off_t
            nc.vector.tensor_scalar(x0[:, :T], rf[:, :T], 1.0, None, OP.subtract)
            fxc = s8
            nc.scalar.activation(out=fxc[:, :T], in_=fx[:, :T],
                                 func=AF.Copy, bias=1.0, scale=-1.0)
            nc.vector.tensor_scalar(o_c[:, :T], x0[:, :T], 0.0, lim2_ap,
                                    OP.max, OP.min)      # xc in [0, W-2]
            d = s6
            nc.vector.tensor_tensor(out=d[:, :T], in0=x0[:, :T], in1=o_c[:, :T],
                                    op=OP.subtract)
            a0 = s9
            nc.vector.tensor_scalar(a0[:, :T], d[:, :T], 0.0, None, OP.is_equal)
            am = s10
            nc.vector.tensor_scalar(am[:, :T], d[:, :T], -1.0, None, OP.is_equal)
            ap1 = s11
            nc.vector.tensor_scalar(ap1[:, :T], d[:, :T], 1.0, None, OP.is_equal)
            t1, t2 = s6, off_t
            nc.vector.tensor_tensor(out=t1[:, :T], in0=fxc[:, :T], in1=a0[:, :T], op=OP.mult)
            nc.vector.tensor_tensor(out=t2[:, :T], in0=fx[:, :T], in1=am[:, :T], op=OP.mult)
            nc.vector.tensor_add(out=o_we[:, :T], in0=t1[:, :T], in1=t2[:, :T])
            t3, t4 = s10, s6
            nc.vector.tensor_tensor(out=t3[:, :T], in0=fx[:, :T], in1=a0[:, :T], op=OP.mult)
            nc.vector.tensor_tensor(out=t4[:, :T], in0=fxc[:, :T], in1=ap1[:, :T], op=OP.mult)
            nc.vector.tensor_add(out=o_wo[:, :T], in0=t3[:, :T], in1=t4[:, :T])

        s6, s7, s8, s9 = ct("s6"), ct("s7"), ct("s8"), ct("s9")
        s10, s11 = ct("s10"), ct("s11")
        xc, yc = ct("xc"), ct("yc")
        wxe, wxo = ct("wxe", FP16), ct("wxo", FP16)
        wye_r, wyo_r = ct("wye_r", FP16), ct("wyo_r", FP16)
        axis_chain(ref_f[0], off_f[0], Wf, Wm2,
                   s6, s7, s8, s9, s10, s11, xc, wxe, wxo)
        axis_chain(ref_f[1], off_f[1], Hf, Hm2,
                   s6, s7, s8, s9, s10, s11, yc, wye_r, wyo_r)
        wye, wyo = ct("wye", FP16), ct("wyo", FP16)
        nc.vector.tensor_tensor(out=wye[:, :T], in0=wye_r[:, :T], in1=a_fm[:, :T], op=OP.mult)
        nc.vector.tensor_tensor(out=wyo[:, :T], in0=wyo_r[:, :T], in1=a_fm[:, :T], op=OP.mult)

        corners = []
        for i, (wy, wx) in enumerate(((wye, wxe), (wye, wxo), (wyo, wxe), (wyo, wxo))):
            c = ct(f"c{i}", bufs=2)
            nc.vector.tensor_tensor(out=c[:, :T], in0=wy[:, :T], in1=wx[:, :T], op=OP.mult)
            corners.append(c)

        ix = ct("ix", bufs=2)
        nc.vector.scalar_tensor_tensor(out=ix[:, :T], in0=yc[:, :T], scalar=Wf,
                                       in1=xc[:, :T], op0=OP.mult, op1=OP.add)
        nc.vector.tensor_scalar(ix[:, :T], ix[:, :T], basef, None, OP.add)

        if dbg and fm0 == 0:
            nc.sync.dma_start(out=dbg["d_afm"][:, :T], in_=a_fm[:, :T])
            nc.sync.dma_start(out=dbg["d_it"][0, :, :T], in_=ix[:, :T])

        def run_blocks():
            _run_blocks(fm0, nb, ix, corners)

        return run_blocks

    def _run_blocks(fm0, nb, ix, corners):
        for j in range(nb):
            t0 = fm0 + j * 128
            sl = slice(j * 128, (j + 1) * 128)
            # wrapped+replicated idx tile: desc i (= lp*128 + t) must sit at
            # [i%16, i//16] in each 16-partition group. Achieved by PE-
            # transposing a free-replicated view of ix: psum[r*16+q, f] =
            # ix[f, 16*jj+q]; ACT-copy (cast->int16) into cols lp*8+jj.
            idxw = blk.tile([128, NH, 16, 8], mybir.dt.int16, tag="idxw", bufs=3)
            for jj in range(8):
                rep = bass.AP(tensor=ix.tensor,
                              offset=ix[:].offset + j * 128 + jj * 16,
                              ap=[ix[:].ap[0], [0, 8], [1, 16]])
                rep8 = blk.tile([128, 128], FP32, tag="rep8")
                nc.scalar.copy(out=rep8[:], in_=rep)
                ps = psTP.tile([128, 128], FP32, tag="tp", name="tp")
                nc.tensor.transpose(out=ps[:], in_=rep8[:], identity=ident[:])
                nc.scalar.copy(out=idxw[:, :, :, jj], in_=ps[:])
            W4 = blk.tile([128, 128, 4], FP16, tag="W4")
            for ci, c in enumerate(corners):
                tp128(c[:, sl], W4[:, :, ci])

            if dbg and fm0 == 0:
                nc.sync.dma_start(out=dbg["d_w4"][j, :, :],
                                  in_=W4[:].rearrange("p a b -> p (a b)"))
            ao = blk.tile([128, D], FP32, tag="ao")
            for h in range(NH):
                G2 = gpool.tile([128, 16, 128], FP16, tag="G2", bufs=5)
                nc.gpsimd.dma_gather(
                    out_ap=G2[:],
                    in_ap=vtabS[h * Lx:(h + 1) * Lx, :],
                    idxs_ap=idxw[:, h, :, :],
                    num_idxs=2048, num_idxs_reg=2048, elem_size=4 * HD,
                    single_packet=False)
                if dbg and fm0 == 0 and h == 0:
                    nc.sync.dma_start(out=dbg["d_g2"][j, :, :2048],
                                      in_=G2[:].rearrange("p a b -> p (a b)"))
                w4h = bass.AP(tensor=W4.tensor, offset=W4[:].offset + h * 64,
                              ap=[W4[:].ap[0], [1, 64], [0, 32]])
                GW = gpool.tile([128, 64, 32], FP16, tag="GW")
                gwf = bass.AP(tensor=GW.tensor, offset=GW[:].offset,
                              ap=[GW[:].ap[0], [1, 2048]])
                nc.vector.tensor_tensor(
                    out=gwf, in0=G2[:].rearrange("p a b -> p (a b)"),
                    in1=w4h, op=OP.mult)
                nc.vector.tensor_reduce(
                    out=ao[:, h * HD:(h + 1) * HD],
                    in_=GW[:].transpose([0, 2, 1]), axis=AX.X, op=OP.add)

            aoFM = blk.tile([128, KC, 128], FP16, tag="aoFM")
            for k in range(KC):
                tp128(ao[:, k * 128:(k + 1) * 128], aoFM[:, k, :])
            psO = psMM.tile([128, 512], FP32, tag="mm")
            for k in range(KC):
                nc.tensor.matmul(psO[:, :D], lhsT=aoFM[:, k, :], rhs=Wo_s[k][:],
                                 start=(k == 0), stop=(k == KC - 1))
            s_t = vtmp.tile([128, D], FP32, tag="s_res")
            nc.sync.dma_start(out=s_t[:], in_=io["srcq"][t0:t0 + 128, :])
            x1 = blk.tile([128, D], FP32, tag="x1", bufs=4)
            nc.vector.scalar_tensor_tensor(out=x1[:], in0=psO[:, :D], scalar=1.0,
                                           in1=s_t[:], op0=OP.mult, op1=OP.add)
            if dbg and fm0 == 0:
                nc.sync.dma_start(out=dbg["d_ao"][j, :, :], in_=ao[:])
                nc.sync.dma_start(out=dbg["d_x1"][j, :, :], in_=x1[:])
            _layernorm(nc, blk, x1, g1_rep, be1_rep, "1", eps_t)
            x1_group.append((t0, x1))
            if len(x1_group) == 2:
                _emit_ffn()

    # FFN on a group of 1-2 finished 128-token blocks, straight from SBUF.
    x1_group = []

    def _emit_ffn():
        group = list(x1_group)
        x1_group.clear()
        nt = len(group)
        ST = nt * 128
        xFM = fmp.tile([128, KC, 512], FP16, tag="xFM")
        for jj, (t0g, xt) in enumerate(group):
            for k in range(KC):
                tp128(xt[:, k * 128:(k + 1) * 128],
                      xFM[:, k, jj * 128:(jj + 1) * 128])
        h1FM = fmp.tile([128, MF, 256], FP16, tag="h1FM")
        for m in range(MF):
            psH = psMM.tile([128, 512], FP32, tag="mm")
            for k in range(KC):
                nc.tensor.matmul(psH[:, :ST], lhsT=W1_s[k][:, m * 128:(m + 1) * 128],
                                 rhs=xFM[:, k, :ST], start=(k == 0), stop=(k == KC - 1))
            nc.scalar.activation(out=h1FM[:, m, :ST], in_=psH[:, :ST],
                                 func=AF.Relu, bias=b1c[:, m:m + 1], scale=1.0)
        for jj, (t0g, xt) in enumerate(group):
            psH2 = psMM.tile([128, 512], FP32, tag="mm")
            for k in range(MF):
                nc.tensor.matmul(psH2[:, :D], lhsT=h1FM[:, k, jj * 128:(jj + 1) * 128],
                                 rhs=W2_s[k][:], start=(k == 0), stop=(k == MF - 1))
            x2 = blk.tile([128, D], FP32, tag="x2")
            nc.vector.scalar_tensor_tensor(out=x2[:], in0=psH2[:, :D], scalar=1.0,
                                           in1=xt[:], op0=OP.mult, op1=OP.add)
            _layernorm(nc, blk, x2, g2_rep, be2_rep, "2", eps_t)
            nc.sync.dma_start(out=out_ap[t0g:t0g + 128, :], in_=x2[:])

    pending = None
    for (_fm0, _T) in _fm_blocks(QPx):
        nxt = compute_fm(_fm0, _T)
        if pending is not None:
            pending()
        pending = nxt
    if pending is not None:
        pending()
    if x1_group:
        _emit_ffn()


def _layernorm(nc, pool, x, g_rep, be_rep, tag, eps_t=None):
    stats = pool.tile([128, 6], FP32, tag=f"st{tag}")
    nc.vector.bn_stats(out=stats[:], in_=x[:])
    mv = pool.tile([128, 2], FP32, tag=f"mv{tag}")
    nc.vector.bn_aggr(out=mv[:], in_=stats[:])
    sd = pool.tile([128, 1], FP32, tag=f"sd{tag}")
    nc.scalar.activation(out=sd[:], in_=mv[:, 1:2], func=AF.Sqrt,
                         bias=eps_t[:, 0:1], scale=1.0)
    rstd = pool.tile([128, 1], FP32, tag=f"rs{tag}")
    nc.vector.reciprocal(out=rstd[:], in_=sd[:])
    nc.vector.tensor_scalar(x[:], x[:], mv[:, 0:1], rstd[:, 0:1],
                            OP.subtract, OP.mult)
    nc.vector.tensor_tensor(out=x[:], in0=x[:], in1=g_rep[:], op=OP.mult)
    nc.vector.tensor_tensor(out=x[:], in0=x[:], in1=be_rep[:], op=OP.add)


# ================= host side =================

def _perm_off():
    return np.concatenate([np.arange(0, D, 2), np.arange(1, D, 2)])


def make_host_consts(shapes, inputs):
    lvl = np.cumsum([0] + [h * w for h, w in shapes])[:-1]
    _, _, _, _, LPx = _plan(shapes)
    f_h = np.arange(128) // 16
    f_l = (np.arange(128) // NP) % NL
    Wl = np.array([w for h, w in shapes], np.float32)
    Hl = np.array([h for h, w in shapes], np.float32)
    fconst = np.zeros((128, 7), np.float32)
    fconst[:, 0] = Wl[f_l]
    fconst[:, 1] = Hl[f_l]
    fconst[:, 2] = Wl[f_l] - 1.0
    fconst[:, 3] = Hl[f_l] - 1.0
    fconst[:, 4] = np.asarray(lvl)[f_l]
    fconst[:, 5] = Wl[f_l] - 2.0
    fconst[:, 6] = Hl[f_l] - 2.0
    perm = _perm_off()
    WoffP = np.asarray(inputs["Woff"], np.float32)[:, perm]
    boffP = (np.asarray(inputs["boff"], np.float32)[perm] - 0.5).reshape(2, 128).T.copy()
    hsum = np.zeros((128, NH), np.float32)
    hsum[np.arange(128), f_h] = 1.0
    hsumT = np.ascontiguousarray(hsum.T)
    b1c = np.asarray(inputs["b1"], np.float32).reshape(DF // 128, 128).T.copy()
    ba_f = np.asarray(inputs["ba"], np.float32).reshape(128, 1).copy()
    pred = np.asarray(inputs["dataset_group_pred"], np.float32)
    lns = {}
    for b in range(pred.shape[0]):
        lns[b] = dict(
            g1=np.ascontiguousarray(pred[b] @ np.asarray(inputs["gw1"], np.float32)),
            be1=np.ascontiguousarray(pred[b] @ np.asarray(inputs["gb1"], np.float32)),
            g2=np.ascontiguousarray(pred[b] @ np.asarray(inputs["gw2"], np.float32)),
            be2=np.ascontiguousarray(pred[b] @ np.asarray(inputs["gb2"], np.float32)),
        )
    return dict(fconst=fconst, WoffP=WoffP, boffP=boffP, hsum=hsum,
                hsumT=hsumT, b1c=b1c, ba_f=ba_f, lns=lns)


def make_core_inputs(core, inputs, shapes=SHAPES, hc=None):
    Lx, _, Qx, QPx, LPx = _plan(shapes)
    if hc is None:
        hc = make_host_consts(shapes, inputs)
    b, s = core // NSHARD, core % NSHARD
    src = np.asarray(inputs["src"], np.float32)
    pos = np.asarray(inputs["pos"], np.float32)
    ref = np.asarray(inputs["reference_points"], np.float32)

    def padQ(a, extra):
        out = np.zeros((QPx,) + extra, np.float32)
        out[:Qx] = a[b, s * Qx:(s + 1) * Qx]
        return out

    src_full = np.zeros((LPx, D), np.float16)
    src_full[:Lx] = src[b].astype(np.float16)
    refq = padQ(ref, (NL, 2))           # [QP, NL, 2]
    f_l = (np.arange(128) // NP) % NL
    refx_fm = np.ascontiguousarray(refq[:, f_l, 0].T)  # [128, QP]
    refy_fm = np.ascontiguousarray(refq[:, f_l, 1].T)
    f16 = lambda k: np.asarray(inputs[k], np.float32).astype(np.float16)
    return dict(
        src_full=src_full,
        srcq=padQ(src, (D,)),
        posq=padQ(pos, (D,)),
        refx_fm=refx_fm, refy_fm=refy_fm,
        Wvh=f16("Wv"),
        WoffPh=hc["WoffP"].astype(np.float16),
        Wah=f16("Wa"),
        Woh=f16("Wo"),
        W1h=f16("W1"),
        W2h=f16("W2"),
        b1c=hc["b1c"], ba_f=hc["ba_f"], boffP=hc["boffP"],
        fconst=hc["fconst"],
        hsum=hc["hsum"], hsumT=hc["hsumT"],
        g1=hc["lns"][b]["g1"], be1=hc["lns"][b]["be1"],
        g2=hc["lns"][b]["g2"], be2=hc["lns"][b]["be2"],
    )


_PROGRAM = None


def _get_program():
    global _PROGRAM
    if _PROGRAM is None:
        _PROGRAM = build_program()
    return _PROGRAM


def _ensure_ntff_hook():
    """Shim antenv.axon_hooks (absent in this image) and register the
    ctypes NTFF profile hook against the injected libaxon so."""
    import types
    if "antenv.axon_hooks" in sys.modules:
        return
    mod = types.ModuleType("antenv.axon_hooks")
    mod._hook = None
    mod.set_axon_ntff_profile_hook = lambda h: setattr(mod, "_hook", h)
    mod.get_axon_ntff_profile_hook = lambda: mod._hook
    sys.modules["antenv.axon_hooks"] = mod
    try:
        from trn_agent_boot.trn_boot import _ntff_profile_via_ctypes
        mod._hook = _ntff_profile_via_ctypes("/opt/axon/libaxon_pjrt.so")
    except Exception as e:
        print(f"ntff hook registration failed: {e}")


def run(inputs, trace=False):
    if trace:
        _ensure_ntff_hook()
    from concourse.bass_utils import run_bass_kernel_spmd
    nc = _get_program()
    hc = make_host_consts(SHAPES, inputs)
    in_maps = [make_core_inputs(c, inputs, hc=hc) for c in range(NCORES)]
    res = run_bass_kernel_spmd(nc, in_maps, core_ids=list(range(NCORES)),
                               trace=trace)
    out = np.zeros((B, L, D), np.float32)
    for c in range(NCORES):
        b, s = c // NSHARD, c % NSHARD
        out[b, s * Q:(s + 1) * Q] = res.results[c]["out"][:Q]
    return out, res


def kernel(**inputs):
    out, _ = run(inputs, trace=False)
    return out



# revision 25
# speedup vs baseline: 1.1668x; 1.1474x over previous
"""Deformable transformer encoder layer on 8 TRN2 NeuronCores.

Sharding: core c -> (batch b=c//4, quarter s=c%4) of the 19560 query tokens
(padded to 4992 = 39*128). Each core computes the full-batch value table
(redundantly, avoids collectives) and writes the 2x2-stencil-expanded table
vtabS[h, entry=(y,x)] = [v(y,x), v(y,x+1), v(y+1,x), v(y+1,x+1)] (256B fp16
entries) DIRECTLY from SBUF with 4 shifted slot writes (no DRAM->DRAM
expansion pass). Entry coords are clamped to [0, W-2]x[0, H-2] so every
referenced entry has all 4 slot pixels inside its level; slot weights carry
the d = x0-xc in {-1,0,+1} selection plus border-validity zeroing. Bilinear
sampling gathers one 256B stencil entry per (token, head, level, point) with
the Ant indirect DMA. The FFN runs interleaved per 256 tokens straight from
SBUF (no DRAM roundtrip). Matmuls run fp16 x fp16 -> fp32 PSUM; coordinates
and LN run fp32.
"""
import sys, os
sys.path.insert(0, "/opt/trn_rl_repo")

import numpy as np
from contextlib import ExitStack

import concourse.bass as bass
import concourse.tile as tile
from concourse import bacc, mybir
from concourse.masks import make_identity

FP32 = mybir.dt.float32
FP16 = mybir.dt.float16
I32 = mybir.dt.int32
AX = mybir.AxisListType
OP = mybir.AluOpType
AF = mybir.ActivationFunctionType

SHAPES = [(92, 160), (46, 80), (23, 40), (12, 20)]
NH, HD, NL, NP = 8, 32, 4, 4
D = NH * HD
DF = 4 * D
EPS = 1e-5
B = 2
NCORES = 8
NSHARD = 4
FMT = 1024  # FM-block token width
NQUEUES = 1  # SWDGE queues for gathers


def _plan(shapes):
    L = sum(h * w for h, w in shapes)
    lvl_start = np.cumsum([0] + [h * w for h, w in shapes])[:-1].tolist()
    Q = (L + NSHARD - 1) // NSHARD
    QP = ((Q + 127) // 128) * 128
    LP = ((L + 127) // 128) * 128
    return L, lvl_start, Q, QP, LP


L, LVL_START, Q, QP, LP = _plan(SHAPES)    # 19560, ..., 4890, 4992, 19584


def _fm_blocks(T, w=FMT):
    out, t0 = [], 0
    while t0 < T:
        out.append((t0, min(w, T - t0)))
        t0 += w
    return out


def build_program(shapes=SHAPES):
    Lx, lvl_start, Qx, QPx, LPx = _plan(shapes)
    NPIX = NH * LPx

    nc = bacc.Bacc("TRN2", target_bir_lowering=False, debug=False,
                   enable_asserts=False, num_devices=1,
                   num_swdge_queues=NQUEUES)

    def din(name, shape, dt=FP32):
        return nc.dram_tensor(name, list(shape), dt, kind="ExternalInput").ap()

    io = {
        "src_full": din("src_full", [LPx, D], FP16),
        "srcq": din("srcq", [QPx, D], FP16),
        "posq": din("posq", [QPx, D], FP16),
        "refx_fm": din("refx_fm", [128, QPx]),
        "refy_fm": din("refy_fm", [128, QPx]),
        "Wvh": din("Wvh", [D, D], FP16),
        "WoffPh": din("WoffPh", [D, D], FP16),
        "Wah": din("Wah", [D, 128], FP16),
        "Woh": din("Woh", [D, D], FP16),
        "W1h": din("W1h", [D, DF], FP16),
        "W2h": din("W2h", [DF, D], FP16),
        "b1c": din("b1c", [128, DF // 128]),
        "ba_f": din("ba_f", [128, 1]),
        "boffP": din("boffP", [128, 2]),
        "fconst": din("fconst", [128, 7]),
        "hsum": din("hsum", [128, NH], FP16),
        "hsumT": din("hsumT", [NH, 128], FP16),
        "g1": din("g1", [D]), "be1": din("be1", [D]),
        "g2": din("g2", [D]), "be2": din("be2", [D]),
    }
    out_ap = nc.dram_tensor("out", [QPx, D], FP32, kind="ExternalOutput").ap()
    dbg = {}
    if os.environ.get("KDBG"):
        dbg = {
            "d_w4": nc.dram_tensor("d_w4", [8, 128, 512], FP16, kind="ExternalOutput").ap(),
            "d_g2": nc.dram_tensor("d_g2", [8, 128, 4096], FP16, kind="ExternalOutput").ap(),
            "d_ao": nc.dram_tensor("d_ao", [8, 128, 256], FP32, kind="ExternalOutput").ap(),
            "d_x1": nc.dram_tensor("d_x1", [8, 128, 256], FP32, kind="ExternalOutput").ap(),
            "d_afm": nc.dram_tensor("d_afm", [128, 1024], FP16, kind="ExternalOutput").ap(),
            "d_it": nc.dram_tensor("d_it", [2, 128, 1024], FP32, kind="ExternalOutput").ap(),
            "d_vtabS": nc.dram_tensor("d_vtabS", [2048, 128], FP16, kind="ExternalOutput").ap(),
        }
    vtabS = nc.dram_tensor("vtabS", [NH * Lx, 4 * HD], FP16).ap()

    with tile.TileContext(nc) as tc:
        with ExitStack() as ctx:
            _body(ctx, tc, io, out_ap, vtabS,
                  shapes, lvl_start, Lx, QPx, LPx, dbg)
    nc.compile()
    return nc


def _body(ctx, tc, io, out_ap, vtabS, shapes, lvl_start, Lx, QPx, LPx, dbg={}):
    nc = tc.nc
    KC = D // 128
    MF = DF // 128

    consts = ctx.enter_context(tc.tile_pool(name="consts", bufs=1))
    vpool = ctx.enter_context(tc.tile_pool(name="vpool", bufs=2))
    vtmp = ctx.enter_context(tc.tile_pool(name="vtmp", bufs=2))
    fmp = ctx.enter_context(tc.tile_pool(name="fmp", bufs=1))
    chain = ctx.enter_context(tc.tile_pool(name="chain", bufs=1))
    blk = ctx.enter_context(tc.tile_pool(name="blk", bufs=2))
    gpool = ctx.enter_context(tc.tile_pool(name="gpool", bufs=2))
    psTP = ctx.enter_context(tc.tile_pool(name="psTP", bufs=2, space="PSUM"))
    psMM = ctx.enter_context(tc.tile_pool(name="psMM", bufs=3, space="PSUM"))
    psS = ctx.enter_context(tc.tile_pool(name="psS", bufs=1, space="PSUM"))

    # ---- constants ----
    ident = consts.tile([128, 128], FP32)
    make_identity(nc, ident[:])
    identh = consts.tile([128, 128], FP16, tag="identh")
    make_identity(nc, identh[:])

    def wtiles(name, ap, ncol):
        ts = []
        for k in range(ap.shape[0] // 128):
            t = consts.tile([128, ncol], FP16, tag=f"{name}{k}")
            nc.sync.dma_start(out=t[:], in_=ap[k * 128:(k + 1) * 128, :])
            ts.append(t)
        return ts

    Wv_s = wtiles("Wv", io["Wvh"], D)
    Woff_s = wtiles("Woff", io["WoffPh"], D)
    Wa_s = wtiles("Wa", io["Wah"], 128)
    Wo_s = wtiles("Wo", io["Woh"], D)
    W1_s = wtiles("W1", io["W1h"], DF)
    W2_s = wtiles("W2", io["W2h"], D)

    def rep_tile(name, ap):
        t = consts.tile([128, D], FP32, tag=name)
        bap = bass.AP(tensor=ap.tensor, offset=ap.offset, ap=[[0, 128], [1, D]])
        nc.gpsimd.dma_start(out=t[:], in_=bap)
        return t

    g1_rep = rep_tile("g1_rep", io["g1"])
    be1_rep = rep_tile("be1_rep", io["be1"])
    g2_rep = rep_tile("g2_rep", io["g2"])
    be2_rep = rep_tile("be2_rep", io["be2"])

    def load(name, ap, dt=FP32):
        t = consts.tile(list(ap.shape), dt, tag=name)
        nc.sync.dma_start(out=t[tuple(slice(0, s) for s in ap.shape)],
                          in_=ap[tuple(slice(None) for _ in ap.shape)])
        return t

    eps_t = consts.tile([128, 1], FP32, tag="eps_t")
    nc.vector.memset(eps_t[:], EPS)
    b1c = load("b1c", io["b1c"])
    ba_f = load("ba_f", io["ba_f"])
    boffP = load("boffP", io["boffP"])
    fconst = load("fconst", io["fconst"])
    hsum = load("hsum", io["hsum"], FP16)
    hsumT = load("hsumT", io["hsumT"], FP16)
    Wf, Hf = fconst[:, 0:1], fconst[:, 1:2]
    Wm1, Hm1 = fconst[:, 2:3], fconst[:, 3:4]
    basef = fconst[:, 4:5]
    Wm2, Hm2 = fconst[:, 5:6], fconst[:, 6:7]

    def tp128(src_ap, dst_ap, engine=None):
        assert src_ap.dtype == FP32
        ps = psTP.tile([128, 128], FP32, tag="tp", name="tp")
        nc.tensor.transpose(out=ps[:], in_=src_ap, identity=ident[:])
        if engine is nc.vector:
            nc.vector.tensor_copy(out=dst_ap, in_=ps[:])
        else:
            nc.scalar.copy(out=dst_ap, in_=ps[:])

    # ================= Phase V: value table =================
    # vtabS entry e=(y,x) of (head, level) holds the 2x2 pixel block as 4
    # slots [v(y,x), v(y,x+1), v(y+1,x), v(y+1,x+1)], written directly from
    # the SBUF value tiles as 4 shifted strided writes (slot (a,b) of entry
    # e is pixel e + a*W + b). Gathers only touch entries with y<=H-2,
    # x<=W-2, whose 4 slot pixels all lie inside the level, so every
    # referenced byte is written; unreferenced entries may hold garbage.
    def tp128h(src_ap, dst_ap):
        ps = psTP.tile([128, 128], FP16, tag="tph", name="tph")
        nc.tensor.transpose(out=ps[:], in_=src_ap, identity=identh[:])
        nc.scalar.copy(out=dst_ap, in_=ps[:])

    eng_rr = [nc.sync, nc.scalar]
    for sup in range(0, LPx, 2048):
        ntile = min(16, (LPx - sup) // 128)
        sup1 = sup + ntile * 128
        v_s = vpool.tile([128, 16, D], FP16, tag="v_s")
        for j in range(ntile):
            t0 = sup + j * 128
            s_t = vtmp.tile([128, D], FP16, tag="s_t")
            nc.sync.dma_start(out=s_t[:], in_=io["src_full"][t0:t0 + 128, :])
            sFM = vtmp.tile([128, KC, 128], FP16, tag="sFM")
            for k in range(KC):
                tp128h(s_t[:, k * 128:(k + 1) * 128], sFM[:, k, :])
            psV = psMM.tile([128, 512], FP32, tag="mm")
            for k in range(KC):
                nc.tensor.matmul(psV[:, :D], lhsT=sFM[:, k, :], rhs=Wv_s[k][:],
                                 start=(k == 0), stop=(k == KC - 1))
            nc.vector.tensor_copy(out=v_s[:, j, :], in_=psV[:, :D])
        ei = 0
        for li, (Hl, Wl) in enumerate(shapes):
            st = lvl_start[li]
            en = st + Hl * Wl
            for a in range(2):
                for b in range(2):
                    p0 = max(sup, st + a * Wl + b)
                    p1 = min(sup1, en)
                    if p1 <= p0:
                        continue
                    pieces = []
                    rel0, rel1 = p0 - sup, p1 - sup
                    if rel0 % 128:
                        c = min(128 - rel0 % 128, rel1 - rel0)
                        pieces.append((rel0, c, 1))
                        rel0 += c
                    nfull = (rel1 - rel0) // 128
                    if nfull:
                        pieces.append((rel0, 128, nfull))
                        rel0 += nfull * 128
                    if rel1 > rel0:
                        pieces.append((rel0, rel1 - rel0, 1))
                    for (rel, cnt, nj) in pieces:
                        j0, pp = rel // 128, rel % 128
                        e0 = sup + rel - st - a * Wl - b
                        for h in range(NH):
                            dst = bass.AP(
                                tensor=vtabS.tensor,
                                offset=((h * Lx + st + e0) * 4
                                        + (a * 2 + b)) * HD,
                                ap=[[4 * HD, cnt], [128 * 4 * HD, nj],
                                    [1, HD]])
                            src = v_s[pp:pp + cnt, j0:j0 + nj,
                                      h * HD:(h + 1) * HD]
                            eng_rr[ei % 2].dma_start(out=dst, in_=src)
                            ei += 1

    if dbg:
        nc.sync.dma_start(out=dbg["d_vtabS"][:, :], in_=vtabS[0:2048, :])

    # ================= Phase M: main =================
    def compute_fm(fm0, T):
        nb = T // 128
        halves = [(n0, min(512, T - n0)) for n0 in range(0, T, 512)]

        def ct(tag, dt=FP32, bufs=None):
            if bufs:
                return chain.tile([128, FMT], dt, tag=tag, name=tag, bufs=bufs)
            return chain.tile([128, FMT], dt, tag=tag, name=tag)

        qFM = fmp.tile([128, KC, FMT], FP16, tag="qFM")
        for j in range(nb):
            t0 = fm0 + j * 128
            qt = fmp.tile([128, D], FP16, tag="q_t", name="q_t", bufs=3)
            pt = vtmp.tile([128, D], FP16, tag="pos_t")
            nc.sync.dma_start(out=qt[:], in_=io["srcq"][t0:t0 + 128, :])
            nc.sync.dma_start(out=pt[:], in_=io["posq"][t0:t0 + 128, :])
            nc.vector.tensor_add(out=qt[:], in0=qt[:], in1=pt[:])
            for k in range(KC):
                tp128h(qt[:, k * 128:(k + 1) * 128], qFM[:, k, j * 128:(j + 1) * 128])

        # offsets FM (boffP includes the -0.5)
        off_f = [ct("s0"), ct("s1")]
        for xy in range(2):
            for (n0, nn) in halves:
                ps = psMM.tile([128, 512], FP32, tag="mm")
                for k in range(KC):
                    nc.tensor.matmul(
                        ps[:, :nn], lhsT=Woff_s[k][:, xy * 128:(xy + 1) * 128],
                        rhs=qFM[:, k, n0:n0 + nn], start=(k == 0), stop=(k == KC - 1))
                nc.vector.tensor_scalar(off_f[xy][:, n0:n0 + nn], ps[:, :nn],
                                        boffP[:, xy:xy + 1], None, OP.add)

        # attention FM + grouped softmax (no max-sub: logits are O(1))
        expt = ct("s2", FP16)
        a_fm = ct("s3", FP16)
        r8 = chain.tile([NH, FMT], FP16, tag="r8")
        for (n0, nn) in halves:
            ps = psMM.tile([128, 512], FP32, tag="mm")
            for k in range(KC):
                nc.tensor.matmul(ps[:, :nn], lhsT=Wa_s[k][:],
                                 rhs=qFM[:, k, n0:n0 + nn],
                                 start=(k == 0), stop=(k == KC - 1))
            nc.scalar.activation(out=expt[:, n0:n0 + nn], in_=ps[:, :nn],
                                 func=AF.Exp, bias=ba_f[:, 0:1], scale=1.0)
            ps8 = psS.tile([NH, 512], FP32, tag="s8")
            nc.tensor.matmul(ps8[:, :nn], lhsT=hsum[:], rhs=expt[:, n0:n0 + nn],
                             start=True, stop=True)
            with nc.allow_low_precision(reason="softmax denom fp16"):
                nc.vector.reciprocal(out=r8[:, n0:n0 + nn], in_=ps8[:, :nn])
            psr = psMM.tile([128, 512], FP32, tag="mm")
            nc.tensor.matmul(psr[:, :nn], lhsT=hsumT[:NH, :], rhs=r8[:, n0:n0 + nn],
                             start=True, stop=True)
            nc.vector.tensor_tensor(out=a_fm[:, n0:n0 + nn],
                                    in0=expt[:, n0:n0 + nn], in1=psr[:, :nn],
                                    op=OP.mult)

        # reference points FM (replicated over heads/points)
        ref_f = []
        for xy, nm in ((0, "refx_fm"), (1, "refy_fm")):
            rt = ct(f"s{4 + xy}")
            nc.sync.dma_start(out=rt[:, :T], in_=io[nm][:, fm0:fm0 + T])
            ref_f.append(rt)

        # ---- FM coordinate / weight / index chain ----
        # Stencil gather: one idx per sample = base + yc*W + xc where
        # xc = clamp(floor(x), 0, W-2), yc likewise, so the 2x2 entry block
        # always lies inside the level. d = floor(x) - xc in {-1, 0, +1}
        # selects which slot holds pixel x0 / x0+1; out-of-image corners get
        # weight 0 (d==+2 when x0 = W matches nothing). Floor is cast-
        # rounding-mode independent: r = float(int(v)); r -= (r > v).
        def axis_chain(ref_t, off_t, scale_ap, lim2_ap,
                       s6, s7, s8, s9, s10, s11, o_c, o_we, o_wo):
            x = off_t
            nc.vector.scalar_tensor_tensor(out=x[:, :T], in0=ref_t[:, :T],
                                           scalar=scale_ap, in1=off_t[:, :T],
                                           op0=OP.mult, op1=OP.add)
            xcl = ref_t
            nc.vector.tensor_scalar(xcl[:, :T], x[:, :T], -1.0, scale_ap,
                                    OP.max, OP.min)
            xp1 = off_t
            nc.vector.tensor_scalar(xp1[:, :T], xcl[:, :T], 1.0, None, OP.add)
            xi = chain.tile([128, FMT], I32, tag="si", name="si")
            nc.vector.tensor_copy(out=xi[:, :T], in_=xp1[:, :T])
            rf = s6
            nc.vector.tensor_copy(out=rf[:, :T], in_=xi[:, :T])
            g = s7
            nc.vector.tensor_tensor(out=g[:, :T], in0=rf[:, :T], in1=xp1[:, :T],
                                    op=OP.is_gt)
            nc.vector.tensor_tensor(out=rf[:, :T], in0=rf[:, :T], in1=g[:, :T],
                                    op=OP.subtract)      # rf = floor(xp1)
            fx = s7
            nc.vector.tensor_tensor(out=fx[:, :T], in0=xp1[:, :T], in1=rf[:, :T],
                                    op=OP.subtract)
            x0 = off_t
            nc.vector.tensor_scalar(x0[:, :T], rf[:, :T], 1.0, None, OP.subtract)
            fxc = s8
            nc.scalar.activation(out=fxc[:, :T], in_=fx[:, :T],
                                 func=AF.Copy, bias=1.0, scale=-1.0)
            nc.vector.tensor_scalar(o_c[:, :T], x0[:, :T], 0.0, lim2_ap,
                                    OP.max, OP.min)      # xc in [0, W-2]
            d = s6
            nc.vector.tensor_tensor(out=d[:, :T], in0=x0[:, :T], in1=o_c[:, :T],
                                    op=OP.subtract)
            a0 = s9
            nc.vector.tensor_scalar(a0[:, :T], d[:, :T], 0.0, None, OP.is_equal)
            am = s10
            nc.vector.tensor_scalar(am[:, :T], d[:, :T], -1.0, None, OP.is_equal)
            ap1 = s11
            nc.vector.tensor_scalar(ap1[:, :T], d[:, :T], 1.0, None, OP.is_equal)
            t1, t2 = s6, off_t
            nc.vector.tensor_tensor(out=t1[:, :T], in0=fxc[:, :T], in1=a0[:, :T], op=OP.mult)
            nc.vector.tensor_tensor(out=t2[:, :T], in0=fx[:, :T], in1=am[:, :T], op=OP.mult)
            nc.vector.tensor_add(out=o_we[:, :T], in0=t1[:, :T], in1=t2[:, :T])
            t3, t4 = s10, s6
            nc.vector.tensor_tensor(out=t3[:, :T], in0=fx[:, :T], in1=a0[:, :T], op=OP.mult)
            nc.vector.tensor_tensor(out=t4[:, :T], in0=fxc[:, :T], in1=ap1[:, :T], op=OP.mult)
            nc.vector.tensor_add(out=o_wo[:, :T], in0=t3[:, :T], in1=t4[:, :T])

        s6, s7, s8, s9 = ct("s6"), ct("s7"), ct("s8"), ct("s9", FP16)
        s10, s11 = ct("s10", FP16), ct("s11", FP16)
        xc, yc = ct("xc"), ct("yc")
        wxe, wxo = ct("wxe", FP16), ct("wxo", FP16)
        wye_r, wyo_r = ct("wye_r", FP16), ct("wyo_r", FP16)
        axis_chain(ref_f[0], off_f[0], Wf, Wm2,
                   s6, s7, s8, s9, s10, s11, xc, wxe, wxo)
        axis_chain(ref_f[1], off_f[1], Hf, Hm2,
                   s6, s7, s8, s9, s10, s11, yc, wye_r, wyo_r)
        wye, wyo = ct("wye", FP16), ct("wyo", FP16)
        nc.vector.tensor_tensor(out=wye[:, :T], in0=wye_r[:, :T], in1=a_fm[:, :T], op=OP.mult)
        nc.vector.tensor_tensor(out=wyo[:, :T], in0=wyo_r[:, :T], in1=a_fm[:, :T], op=OP.mult)

        corners = []
        for i, (wy, wx) in enumerate(((wye, wxe), (wye, wxo), (wyo, wxe), (wyo, wxo))):
            c = ct(f"c{i}", bufs=2)
            nc.vector.tensor_tensor(out=c[:, :T], in0=wy[:, :T], in1=wx[:, :T], op=OP.mult)
            corners.append(c)

        ix = ct("ix", bufs=2)
        nc.vector.scalar_tensor_tensor(out=ix[:, :T], in0=yc[:, :T], scalar=Wf,
                                       in1=xc[:, :T], op0=OP.mult, op1=OP.add)
        nc.vector.tensor_scalar(ix[:, :T], ix[:, :T], basef, None, OP.add)

        if dbg and fm0 == 0:
            nc.sync.dma_start(out=dbg["d_afm"][:, :T], in_=a_fm[:, :T])
            nc.sync.dma_start(out=dbg["d_it"][0, :, :T], in_=ix[:, :T])

        return dict(fm0=fm0, nb=nb, ix=ix, corners=corners)

    def prep_block(info, j):
        # idxw + W4 for block j, emitted one block AHEAD of its gathers so
        # ACT/PE build them while the previous block's gathers run.
        ix, corners = info["ix"], info["corners"]
        sl = slice(j * 128, (j + 1) * 128)
        # wrapped+replicated idx tile: desc i (= lp*128 + t) must sit at
        # [i%16, i//16] in each 16-partition group. Achieved by PE-
        # transposing a free-replicated view of ix: psum[r*16+q, f] =
        # ix[f, 16*jj+q]; ACT-copy (cast->int16) into cols lp*8+jj.
        idxw = blk.tile([128, NH, 16, 8], mybir.dt.int16, tag="idxw", bufs=3)
        for jj in range(8):
            rep = bass.AP(tensor=ix.tensor,
                          offset=ix[:].offset + j * 128 + jj * 16,
                          ap=[ix[:].ap[0], [0, 8], [1, 16]])
            rep8 = blk.tile([128, 128], FP32, tag="rep8", bufs=3)
            nc.scalar.copy(out=rep8[:], in_=rep)
            ps = psTP.tile([128, 128], FP32, tag="tp", name="tp")
            nc.tensor.transpose(out=ps[:], in_=rep8[:], identity=ident[:])
            nc.scalar.copy(out=idxw[:, :, :, jj], in_=ps[:])
        W4 = blk.tile([128, 128, 4], FP16, tag="W4", bufs=3)
        for ci, c in enumerate(corners):
            tp128(c[:, sl], W4[:, :, ci])
        return idxw, W4

    def consume_block(info, j, idxw, W4):
        fm0 = info["fm0"]
        t0 = fm0 + j * 128
        if dbg and fm0 == 0:
            nc.sync.dma_start(out=dbg["d_w4"][j, :, :],
                              in_=W4[:].rearrange("p a b -> p (a b)"))
        ao = blk.tile([128, D], FP32, tag="ao")
        for h in range(NH):
            G2 = gpool.tile([128, 16, 128], FP16, tag="G2", bufs=6)
            nc.gpsimd.dma_gather(
                out_ap=G2[:],
                in_ap=vtabS[h * Lx:(h + 1) * Lx, :],
                idxs_ap=idxw[:, h, :, :],
                num_idxs=2048, num_idxs_reg=2048, elem_size=4 * HD,
                single_packet=False)
            if dbg and fm0 == 0 and h == 0:
                nc.sync.dma_start(out=dbg["d_g2"][j, :, :2048],
                                  in_=G2[:].rearrange("p a b -> p (a b)"))
            w4h = bass.AP(tensor=W4.tensor, offset=W4[:].offset + h * 64,
                          ap=[W4[:].ap[0], [1, 64], [0, 32]])
            GW = gpool.tile([128, 64, 32], FP16, tag="GW")

            def gwsl(start, n):
                return bass.AP(tensor=GW.tensor, offset=GW[:].offset + start,
                               ap=[GW[:].ap[0], [1, n]])

            nc.vector.tensor_tensor(
                out=gwsl(0, 2048), in0=G2[:].rearrange("p a b -> p (a b)"),
                in1=w4h, op=OP.mult)
            # contiguous halving tree over the 64 (sample, slot) groups
            with nc.allow_low_precision(reason="bilinear partial sums fp16"):
                n = 1024
                while n > 32:
                    nc.vector.tensor_add(out=gwsl(0, n), in0=gwsl(0, n),
                                         in1=gwsl(n, n))
                    n //= 2
            nc.vector.tensor_add(out=ao[:, h * HD:(h + 1) * HD],
                                 in0=gwsl(0, 32), in1=gwsl(32, 32))

        aoFM = blk.tile([128, KC, 128], FP16, tag="aoFM")
        for k in range(KC):
            tp128(ao[:, k * 128:(k + 1) * 128], aoFM[:, k, :])
        psO = psMM.tile([128, 512], FP32, tag="mm")
        for k in range(KC):
            nc.tensor.matmul(psO[:, :D], lhsT=aoFM[:, k, :], rhs=Wo_s[k][:],
                             start=(k == 0), stop=(k == KC - 1))
        s_t = vtmp.tile([128, D], FP16, tag="s_res")
        nc.sync.dma_start(out=s_t[:], in_=io["srcq"][t0:t0 + 128, :])
        x1 = blk.tile([128, D], FP32, tag="x1", bufs=4)
        nc.vector.scalar_tensor_tensor(out=x1[:], in0=psO[:, :D], scalar=1.0,
                                       in1=s_t[:], op0=OP.mult, op1=OP.add)
        if dbg and fm0 == 0:
            nc.sync.dma_start(out=dbg["d_ao"][j, :, :], in_=ao[:])
            nc.sync.dma_start(out=dbg["d_x1"][j, :, :], in_=x1[:])
        _layernorm(nc, blk, x1, g1_rep, be1_rep, "1", eps_t)
        x1_group.append((t0, x1))
        if len(x1_group) == 2:
            _emit_ffn()

    # FFN on a group of 1-2 finished 128-token blocks, straight from SBUF.
    x1_group = []

    def _emit_ffn():
        group = list(x1_group)
        x1_group.clear()
        nt = len(group)
        ST = nt * 128
        xFM = fmp.tile([128, KC, 512], FP16, tag="xFM")
        for jj, (t0g, xt) in enumerate(group):
            for k in range(KC):
                tp128(xt[:, k * 128:(k + 1) * 128],
                      xFM[:, k, jj * 128:(jj + 1) * 128])
        h1FM = fmp.tile([128, MF, 256], FP16, tag="h1FM")
        for m in range(MF):
            psH = psMM.tile([128, 512], FP32, tag="mm")
            for k in range(KC):
                nc.tensor.matmul(psH[:, :ST], lhsT=W1_s[k][:, m * 128:(m + 1) * 128],
                                 rhs=xFM[:, k, :ST], start=(k == 0), stop=(k == KC - 1))
            nc.scalar.activation(out=h1FM[:, m, :ST], in_=psH[:, :ST],
                                 func=AF.Relu, bias=b1c[:, m:m + 1], scale=1.0)
        for jj, (t0g, xt) in enumerate(group):
            psH2 = psMM.tile([128, 512], FP32, tag="mm")
            for k in range(MF):
                nc.tensor.matmul(psH2[:, :D], lhsT=h1FM[:, k, jj * 128:(jj + 1) * 128],
                                 rhs=W2_s[k][:], start=(k == 0), stop=(k == MF - 1))
            x2 = blk.tile([128, D], FP32, tag="x2")
            nc.vector.scalar_tensor_tensor(out=x2[:], in0=psH2[:, :D], scalar=1.0,
                                           in1=xt[:], op0=OP.mult, op1=OP.add)
            _layernorm(nc, blk, x2, g2_rep, be2_rep, "2", eps_t)
            nc.sync.dma_start(out=out_ap[t0g:t0g + 128, :], in_=x2[:])

    # Driver: software-pipelined. Chain k+1 is emitted two blocks into fm k's
    # consumption (DVE slack absorbs the lump), and each block's idxw/W4 prep
    # is emitted one block ahead of its gathers.
    fmlist = _fm_blocks(QPx)
    blockof = []

    def emit_chain(k):
        info = compute_fm(*fmlist[k])
        for j in range(info["nb"]):
            blockof.append((info, j))

    preps = {}

    def prep_bi(i):
        if i < len(blockof) and i not in preps:
            preps[i] = prep_block(*blockof[i])

    emit_chain(0)
    prep_bi(0)
    gi = 0
    for k in range(len(fmlist)):
        nb_k = fmlist[k][1] // 128
        for jl in range(nb_k):
            if jl == 2 and k + 1 < len(fmlist):
                emit_chain(k + 1)
            prep_bi(gi + 1)
            info, j = blockof[gi]
            idxw, W4 = preps.pop(gi)
            consume_block(info, j, idxw, W4)
            gi += 1
    if x1_group:
        _emit_ffn()


def _layernorm(nc, pool, x, g_rep, be_rep, tag, eps_t=None):
    stats = pool.tile([128, 6], FP32, tag=f"st{tag}")
    nc.vector.bn_stats(out=stats[:], in_=x[:])
    mv = pool.tile([128, 2], FP32, tag=f"mv{tag}")
    nc.vector.bn_aggr(out=mv[:], in_=stats[:])
    sd = pool.tile([128, 1], FP32, tag=f"sd{tag}")
    nc.scalar.activation(out=sd[:], in_=mv[:, 1:2], func=AF.Sqrt,
                         bias=eps_t[:, 0:1], scale=1.0)
    rstd = pool.tile([128, 1], FP32, tag=f"rs{tag}")
    nc.vector.reciprocal(out=rstd[:], in_=sd[:])
    nc.vector.tensor_scalar(x[:], x[:], mv[:, 0:1], rstd[:, 0:1],
                            OP.subtract, OP.mult)
    nc.vector.tensor_tensor(out=x[:], in0=x[:], in1=g_rep[:], op=OP.mult)
    nc.vector.tensor_tensor(out=x[:], in0=x[:], in1=be_rep[:], op=OP.add)


# ================= host side =================

def _perm_off():
    return np.concatenate([np.arange(0, D, 2), np.arange(1, D, 2)])


def make_host_consts(shapes, inputs):
    lvl = np.cumsum([0] + [h * w for h, w in shapes])[:-1]
    _, _, _, _, LPx = _plan(shapes)
    f_h = np.arange(128) // 16
    f_l = (np.arange(128) // NP) % NL
    Wl = np.array([w for h, w in shapes], np.float32)
    Hl = np.array([h for h, w in shapes], np.float32)
    fconst = np.zeros((128, 7), np.float32)
    fconst[:, 0] = Wl[f_l]
    fconst[:, 1] = Hl[f_l]
    fconst[:, 2] = Wl[f_l] - 1.0
    fconst[:, 3] = Hl[f_l] - 1.0
    fconst[:, 4] = np.asarray(lvl)[f_l]
    fconst[:, 5] = Wl[f_l] - 2.0
    fconst[:, 6] = Hl[f_l] - 2.0
    perm = _perm_off()
    WoffP = np.asarray(inputs["Woff"], np.float32)[:, perm]
    boffP = (np.asarray(inputs["boff"], np.float32)[perm] - 0.5).reshape(2, 128).T.copy()
    hsum = np.zeros((128, NH), np.float32)
    hsum[np.arange(128), f_h] = 1.0
    hsumT = np.ascontiguousarray(hsum.T)
    b1c = np.asarray(inputs["b1"], np.float32).reshape(DF // 128, 128).T.copy()
    ba_f = np.asarray(inputs["ba"], np.float32).reshape(128, 1).copy()
    pred = np.asarray(inputs["dataset_group_pred"], np.float32)
    lns = {}
    for b in range(pred.shape[0]):
        lns[b] = dict(
            g1=np.ascontiguousarray(pred[b] @ np.asarray(inputs["gw1"], np.float32)),
            be1=np.ascontiguousarray(pred[b] @ np.asarray(inputs["gb1"], np.float32)),
            g2=np.ascontiguousarray(pred[b] @ np.asarray(inputs["gw2"], np.float32)),
            be2=np.ascontiguousarray(pred[b] @ np.asarray(inputs["gb2"], np.float32)),
        )
    return dict(fconst=fconst, WoffP=WoffP, boffP=boffP,
                hsum=hsum.astype(np.float16),
                hsumT=hsumT.astype(np.float16), b1c=b1c, ba_f=ba_f, lns=lns)


def make_core_inputs(core, inputs, shapes=SHAPES, hc=None):
    Lx, _, Qx, QPx, LPx = _plan(shapes)
    if hc is None:
        hc = make_host_consts(shapes, inputs)
    b, s = core // NSHARD, core % NSHARD
    src = np.asarray(inputs["src"], np.float32)
    pos = np.asarray(inputs["pos"], np.float32)
    ref = np.asarray(inputs["reference_points"], np.float32)

    def padQ(a, extra):
        out = np.zeros((QPx,) + extra, np.float32)
        out[:Qx] = a[b, s * Qx:(s + 1) * Qx]
        return out

    src_full = np.zeros((LPx, D), np.float16)
    src_full[:Lx] = src[b].astype(np.float16)
    refq = padQ(ref, (NL, 2))           # [QP, NL, 2]
    f_l = (np.arange(128) // NP) % NL
    refx_fm = np.ascontiguousarray(refq[:, f_l, 0].T)  # [128, QP]
    refy_fm = np.ascontiguousarray(refq[:, f_l, 1].T)
    f16 = lambda k: np.asarray(inputs[k], np.float32).astype(np.float16)
    return dict(
        src_full=src_full,
        srcq=padQ(src, (D,)).astype(np.float16),
        posq=padQ(pos, (D,)).astype(np.float16),
        refx_fm=refx_fm, refy_fm=refy_fm,
        Wvh=f16("Wv"),
        WoffPh=hc["WoffP"].astype(np.float16),
        Wah=f16("Wa"),
        Woh=f16("Wo"),
        W1h=f16("W1"),
        W2h=f16("W2"),
        b1c=hc["b1c"], ba_f=hc["ba_f"], boffP=hc["boffP"],
        fconst=hc["fconst"],
        hsum=hc["hsum"], hsumT=hc["hsumT"],
        g1=hc["lns"][b]["g1"], be1=hc["lns"][b]["be1"],
        g2=hc["lns"][b]["g2"], be2=hc["lns"][b]["be2"],
    )


_PROGRAM = None


def _get_program():
    global _PROGRAM
    if _PROGRAM is None:
        _PROGRAM = build_program()
    return _PROGRAM


def _ensure_ntff_hook():
    """Shim antenv.axon_hooks (absent in this image) and register the
    ctypes NTFF profile hook against the injected libaxon so."""
    import types
    if "antenv.axon_hooks" in sys.modules:
        return
    mod = types.ModuleType("antenv.axon_hooks")
    mod._hook = None
    mod.set_axon_ntff_profile_hook = lambda h: setattr(mod, "_hook", h)
    mod.get_axon_ntff_profile_hook = lambda: mod._hook
    sys.modules["antenv.axon_hooks"] = mod
    try:
        from trn_agent_boot.trn_boot import _ntff_profile_via_ctypes
        mod._hook = _ntff_profile_via_ctypes("/opt/axon/libaxon_pjrt.so")
    except Exception as e:
        print(f"ntff hook registration failed: {e}")


def run(inputs, trace=False):
    if trace:
        _ensure_ntff_hook()
    from concourse.bass_utils import run_bass_kernel_spmd
    nc = _get_program()
    hc = make_host_consts(SHAPES, inputs)
    in_maps = [make_core_inputs(c, inputs, hc=hc) for c in range(NCORES)]
    res = run_bass_kernel_spmd(nc, in_maps, core_ids=list(range(NCORES)),
                               trace=trace)
    out = np.zeros((B, L, D), np.float32)
    for c in range(NCORES):
        b, s = c // NSHARD, c % NSHARD
        out[b, s * Q:(s + 1) * Q] = res.results[c]["out"][:Q]
    return out, res


def kernel(**inputs):
    out, _ = run(inputs, trace=False)
    return out

